# revision 10
# baseline (speedup 1.0000x reference)
"""ClassicalSelfAttention TRN2 kernel — 8-core SPMD, sequence-parallel.

out = softmax((X Wq)(X Wk)^T / sqrt(d)) @ X,  X:[4096,1024] f32, W:[1024,1024].

Per core (rows sharded 8x512), using S_l = ((Xl Wq) Wk^T) X^T so no
replicated projection work:
  Q^T = Wq^T Xl^T        fp16x2 (hh+hl+lh), lhsT = Wq natural layout
  C^T = Wk   Q^T         fp16x2, lhsT = Wk^T (host-transposed)
  S   = C    X^T         fp16x2, rhs = X^T (host-transposed + host-split)
  P   = softmax(S/32)    2-pass; S stored fp16 as (S - chunkmax)*scale
  out = (P @ X) * 1/rowsum   single-pass fp16 (P^T via PE transpose)

Host prep inside kernel(): transpose + fp16 hi/lo split of X / X^T / Wq /
Wk^T (layout-only work; all FLOPs of the computation run on device).
Logits need ~fp32 precision (top-2 gaps down to 3e-3 post-scale; bf16 or
f32r single-pass flips argmax rows), hence fp16x2 for the whole S chain.
P/V tolerate fp16 single-pass. P^T is consumed tile-by-tile straight out
of PSUM copies, so nothing spills to DRAM.
"""
import os
import numpy as np
import concourse.bass as bass
import concourse.bacc as bacc
import concourse.mybir as mybir
import concourse.tile as tile
from concourse import masks
from concourse.bass_utils import run_bass_kernel_spmd

F32 = mybir.dt.float32
F16 = mybir.dt.float16

D = 1024          # embed dim
NT = 4096         # tokens
NC = 8            # cores
NL = NT // NC     # 512 local rows
ET = D // 128     # 8 embed tiles
JC = NT // 512    # 8 j-chunks
MT = NL // 128    # 4 m-tiles
SCALE = float(1.0 / np.sqrt(np.float32(D)))

EXP = mybir.ActivationFunctionType.Exp
IDENT = mybir.ActivationFunctionType.Identity
AX = mybir.AxisListType.X
SUB = mybir.AluOpType.subtract
MUL = mybir.AluOpType.mult


def _split(nc, psrc, hdst, ldst):
    """psum f32 -> hdst f16 (round) and ldst f16 (residual), both on DVE."""
    nc.vector.tensor_copy(hdst, psrc)
    nc.vector.tensor_sub(ldst, psrc, hdst)


_PHASES = int(os.environ.get("KPHASES", "99"))


def build_nc():
    nc = bacc.Bacc("TRN2", target_bir_lowering=False, debug=False)

    xh_d = nc.declare_dram_parameter("xh", [NT, D], F16, isOutput=False)
    xth_d = nc.declare_dram_parameter("xth", [D, NT], F16, isOutput=False)
    xtl_d = nc.declare_dram_parameter("xtl", [D, NT], F16, isOutput=False)
    xlth_d = nc.declare_dram_parameter("xlth", [D, NL], F16, isOutput=False)
    xltl_d = nc.declare_dram_parameter("xltl", [D, NL], F16, isOutput=False)
    wqh_d = nc.declare_dram_parameter("wqh", [D, D], F16, isOutput=False)
    wql_d = nc.declare_dram_parameter("wql", [D, D], F16, isOutput=False)
    wkth_d = nc.declare_dram_parameter("wkth", [D, D], F16, isOutput=False)
    wktl_d = nc.declare_dram_parameter("wktl", [D, D], F16, isOutput=False)
    out_d = nc.declare_dram_parameter("out_local", [NL, D], F32, isOutput=True)

    with tile.TileContext(nc) as tc:
        with (
            tc.tile_pool(name="persist", bufs=1) as persist,
            tc.tile_pool(name="stats", bufs=1) as stats,
        ):
            ident16 = persist.tile([128, 128], F16, tag="id16", name="id16")
            masks.make_identity(nc, ident16[:])

            # chunk max / exp-pass bias / exp sums, one [128, JC] per m-tile
            pmax = [stats.tile([128, JC], F32, tag=f"pmax{m}", name=f"pmax{m}") for m in range(MT)]
            bsub = [stats.tile([128, JC], F32, tag=f"bsub{m}", name=f"bsub{m}") for m in range(MT)]
            bexp = [stats.tile([128, JC], F32, tag=f"bexp{m}", name=f"bexp{m}") for m in range(MT)]
            esum = [stats.tile([128, JC], F32, tag=f"esum{m}", name=f"esum{m}") for m in range(MT)]
            recip = stats.tile([128, MT], F32, tag="recip", name="recip")

            cth = persist.tile([128, ET, NL], F16, tag="cth", name="cth")
            ctl = persist.tile([128, ET, NL], F16, tag="ctl", name="ctl")
            s16 = [persist.tile([128, JC, 512], F16, tag=f"s16_{m}", name=f"s16_{m}")
                   for m in range(MT)]
            xh_sb = persist.tile([128, NT // 128, D], F16, tag="xhsb", name="xhsb")

            # ---------------- P1 + P2: Q^T then C^T ----------------
            with tc.tile_pool(name="qt", bufs=1) as qtp:
                qth = qtp.tile([128, ET, NL], F16, tag="qth", name="qth")
                qtl = qtp.tile([128, ET, NL], F16, tag="qtl", name="qtl")

                with (
                    tc.tile_pool(name="w1", bufs=1) as w1,
                    tc.tile_pool(name="psB", bufs=1, space=bass.MemorySpace.PSUM) as psB,
                ):
                    wqh_sb = w1.tile([128, ET, D], F16, tag="wqh", name="wqh")
                    wql_sb = w1.tile([128, ET, D], F16, tag="wql", name="wql")
                    xlth_sb = w1.tile([128, ET, NL], F16, tag="xlth", name="xlth")
                    xltl_sb = w1.tile([128, ET, NL], F16, tag="xltl", name="xltl")
                    # per-es pieces so the first matmuls can start early
                    for es in range(ET):
                        r = slice(es * 128, (es + 1) * 128)
                        nc.sync.dma_start(xlth_sb[:, es, :], xlth_d[r, :])
                        nc.sync.dma_start(xltl_sb[:, es, :], xltl_d[r, :])
                        nc.sync.dma_start(wqh_sb[:, es, :], wqh_d[r, :])
                        nc.sync.dma_start(wql_sb[:, es, :], wql_d[r, :])

                    # es-outer over all 8 dp PSUM banks: matmuls chase the
                    # incoming per-es DMA pieces, hiding the load entirely
                    pqs = [psB.tile([128, NL], F32, tag=f"pq{dp}", name=f"pq{dp}")
                           for dp in range(ET)]
                    for es in range(ET if _PHASES >= 1 else 0):
                        for dp in range(ET):
                            dcol = slice(dp * 128, (dp + 1) * 128)
                            for k, (lh, rh) in enumerate(((wqh_sb, xlth_sb),
                                                          (wqh_sb, xltl_sb),
                                                          (wql_sb, xlth_sb))):
                                nc.tensor.matmul(pqs[dp][:], lh[:, es, dcol],
                                                 rh[:, es, :],
                                                 start=(es == 0 and k == 0),
                                                 stop=(es == ET - 1 and k == 2))
                    if _PHASES >= 1:
                        for dp in range(ET):
                            _split(nc, pqs[dp][:], qth[:, dp, :], qtl[:, dp, :])

                with tc.tile_pool(name="psA", bufs=4, space=bass.MemorySpace.PSUM) as psA:
                    with tc.tile_pool(name="w2", bufs=1) as w2:
                        wkth_sb = w2.tile([128, ET, D], F16, tag="wkth", name="wkth")
                        wktl_sb = w2.tile([128, ET, D], F16, tag="wktl", name="wktl")
                        for ds in range(ET):
                            r = slice(ds * 128, (ds + 1) * 128)
                            nc.sync.dma_start(wkth_sb[:, ds, :], wkth_d[r, :])
                            nc.sync.dma_start(wktl_sb[:, ds, :], wktl_d[r, :])

                        for ep in range(ET if _PHASES >= 2 else 0):
                            pc = psA.tile([128, NL], F32, tag="acc", name="acc")
                            n_mm = 0
                            for ds in range(ET):
                                ecol = slice(ep * 128, (ep + 1) * 128)
                                for lh, rh in ((wkth_sb, qth), (wkth_sb, qtl),
                                               (wktl_sb, qth)):
                                    nc.tensor.matmul(pc[:], lh[:, ds, ecol], rh[:, ds, :],
                                                     start=(n_mm == 0), stop=(n_mm == 23))
                                    n_mm += 1
                            _split(nc, pc[:], cth[:, ep, :], ctl[:, ep, :])

                    # ------------- P3: S chunks, chunk-max, fp16 store -------------
                    with tc.tile_pool(name="stream", bufs=2) as stream:
                        for jc in range(JC if _PHASES >= 3 else 0):
                            cols = slice(jc * 512, (jc + 1) * 512)
                            xch = stream.tile([128, ET, 512], F16, tag="xch", name="xch")
                            xcl = stream.tile([128, ET, 512], F16, tag="xcl", name="xcl")
                            nc.sync.dma_start(
                                xch[:], xth_d[:, cols].rearrange("(es p) j -> p es j", p=128))
                            nc.sync.dma_start(
                                xcl[:], xtl_d[:, cols].rearrange("(es p) j -> p es j", p=128))
                            # V rows for this chunk (used in P5), overlapped here
                            nc.sync.dma_start(
                                xh_sb[:, jc * 4:(jc + 1) * 4, :],
                                xh_d[jc * 512:(jc + 1) * 512, :]
                                .rearrange("(jt p) d -> p jt d", p=128))

                            for m in range(MT):
                                ps = psA.tile([128, 512], F32, tag="acc", name="acc")
                                mcol = slice(m * 128, (m + 1) * 128)
                                n_mm = 0
                                for es in range(ET):
                                    for lh, rh in ((cth, xch), (cth, xcl), (ctl, xch)):
                                        nc.tensor.matmul(ps[:], lh[:, es, mcol], rh[:, es, :],
                                                         start=(n_mm == 0), stop=(n_mm == 23))
                                        n_mm += 1
                                nc.vector.reduce_max(pmax[m][:, jc:jc + 1], ps[:], axis=AX)
                                nc.vector.tensor_scalar_mul(
                                    bsub[m][:, jc:jc + 1], pmax[m][:, jc:jc + 1], -SCALE)
                                nc.scalar.activation(s16[m][:, jc, :], ps[:], IDENT,
                                                     bias=bsub[m][:, jc:jc + 1], scale=SCALE)

            # ---------------- P4: global row max -> exp biases ----------------
            for m in range(MT if _PHASES >= 4 else 0):
                rowmax = stats.tile([128, 1], F32, tag=f"rmax{m}", name=f"rmax{m}")
                nc.vector.reduce_max(rowmax[:], pmax[m][:], axis=AX)
                # bexp = (pmax - rowmax) * SCALE
                nc.vector.tensor_scalar(bexp[m][:], pmax[m][:], rowmax[:], SCALE,
                                        op0=SUB, op1=MUL)

            # ---------------- P5: exp, P^T, P@V, scale ----------------
            with (
                tc.tile_pool(name="pvps", bufs=2, space=bass.MemorySpace.PSUM) as pvps,
                tc.tile_pool(name="ptps", bufs=3, space=bass.MemorySpace.PSUM) as ptps,
                tc.tile_pool(name="p5s", bufs=3) as p5s,
            ):
                def p5_stage(m, jc):
                    """exp -> 4 PE transposes -> one DVE copy; returns P^T tiles."""
                    pch = p5s.tile([128, 512], F16, tag="pch", name="pch")
                    nc.scalar.activation(pch[:], s16[m][:, jc, :], EXP,
                                         bias=bexp[m][:, jc:jc + 1], scale=1.0,
                                         accum_out=esum[m][:, jc:jc + 1])
                    ptt = ptps.tile([128, 4, 128], F16, tag="ptt", name="ptt")
                    for js in range(4):
                        nc.tensor.transpose(ptt[:, js, :],
                                            pch[:, js * 128:(js + 1) * 128], ident16[:])
                    pts = p5s.tile([128, 4, 128], F16, tag="pts", name="pts")
                    nc.vector.tensor_copy(pts[:], ptt[:])
                    return pts

                for m in range(MT if _PHASES >= 5 else 0):
                    pv = pvps.tile([128, 2, 512], F32, tag="pv", name="pv")
                    # stage jc+1 ahead of chunk jc's PV matmuls so the PE
                    # never waits on the PSUM->SBUF copy of its lhsT tiles
                    pts_cur = p5_stage(m, 0)
                    for jc in range(JC):
                        pts_nxt = p5_stage(m, jc + 1) if jc + 1 < JC else None
                        for js in range(4):
                            jt = jc * 4 + js
                            for n in range(2):
                                nc.tensor.matmul(
                                    pv[:, n, :], pts_cur[:, js, :],
                                    xh_sb[:, jt, n * 512:(n + 1) * 512],
                                    start=(jt == 0), stop=(jt == NT // 128 - 1))
                        pts_cur = pts_nxt
                    rs = stats.tile([128, 1], F32, tag=f"rs{m}", name=f"rs{m}")
                    nc.vector.reduce_sum(rs[:], esum[m][:], axis=AX)
                    nc.vector.reciprocal(recip[:, m:m + 1], rs[:])
                    for n in range(2):
                        osb = p5s.tile([128, 512], F32, tag="osb", name="osb")
                        nc.vector.tensor_scalar_mul(osb[:], pv[:, n, :], recip[:, m:m + 1])
                        nc.sync.dma_start(
                            out_d[m * 128:(m + 1) * 128, n * 512:(n + 1) * 512], osb[:])

    nc.compile()
    return nc


_NC_CACHE = None


def _split16(a):
    h = a.astype(np.float16)
    l = (a - h.astype(np.float32)).astype(np.float16)
    return h, l


def kernel(inputs, rotation_params, entangle_params):
    global _NC_CACHE
    if _NC_CACHE is None:
        _NC_CACHE = build_nc()
    nc = _NC_CACHE

    x = np.ascontiguousarray(np.asarray(inputs, np.float32))
    wq = np.ascontiguousarray(np.asarray(rotation_params, np.float32))
    wkt = np.ascontiguousarray(np.asarray(entangle_params, np.float32).T)
    xt = np.ascontiguousarray(x.T)

    xh, _ = _split16(x)
    xth, xtl = _split16(xt)
    wqh, wql = _split16(wq)
    wkth, wktl = _split16(wkt)

    in_maps = []
    for c in range(NC):
        cols = slice(c * NL, (c + 1) * NL)
        in_maps.append({
            "xh": xh, "xth": xth, "xtl": xtl,
            "xlth": np.ascontiguousarray(xth[:, cols]),
            "xltl": np.ascontiguousarray(xtl[:, cols]),
            "wqh": wqh, "wql": wql, "wkth": wkth, "wktl": wktl,
        })
    r = run_bass_kernel_spmd(nc, in_maps, list(range(NC)))
    return np.concatenate([r.results[c]["out_local"] for c in range(NC)], axis=0)


# revision 12
# speedup vs baseline: 1.0627x; 1.0627x over previous
"""ClassicalSelfAttention TRN2 kernel — 8-core SPMD, sequence-parallel.

out = softmax((X Wq)(X Wk)^T / sqrt(d)) @ X,  X:[4096,1024] f32, W:[1024,1024].

Per core (rows sharded 8x512), using S_l = ((Xl Wq) Wk^T) X^T so no
replicated projection work:
  Q^T = Wq^T Xl^T        fp16x2 (hh+hl+lh), lhsT = Wq natural layout
  C^T = Wk   Q^T         fp16x2, lhsT = Wk^T (host-transposed)
  S   = C    X^T         fp16x2, rhs = X^T (host-transposed + host-split)
  P   = softmax(S/32)    2-pass; S stored fp16 as (S - chunkmax)*scale
  out = (P @ X) * 1/rowsum   single-pass fp16 (P^T via PE transpose)

Host prep inside kernel(): transpose + fp16 hi/lo split of X / X^T / Wq /
Wk^T (layout-only work; all FLOPs of the computation run on device).
Logits need ~fp32 precision (top-2 gaps down to 3e-3 post-scale; bf16 or
f32r single-pass flips argmax rows), hence fp16x2 for the whole S chain.
P/V tolerate fp16 single-pass. P^T is consumed tile-by-tile straight out
of PSUM copies, so nothing spills to DRAM.
"""
import os
import numpy as np
import concourse.bass as bass
import concourse.bacc as bacc
import concourse.mybir as mybir
import concourse.tile as tile
from concourse import masks
from concourse.bass_utils import run_bass_kernel_spmd

F32 = mybir.dt.float32
F16 = mybir.dt.float16

D = 1024          # embed dim
NT = 4096         # tokens
NC = 8            # cores
NL = NT // NC     # 512 local rows
ET = D // 128     # 8 embed tiles
JC = NT // 512    # 8 j-chunks
MT = NL // 128    # 4 m-tiles
SCALE = float(1.0 / np.sqrt(np.float32(D)))

EXP = mybir.ActivationFunctionType.Exp
IDENT = mybir.ActivationFunctionType.Identity
AX = mybir.AxisListType.X
SUB = mybir.AluOpType.subtract
MUL = mybir.AluOpType.mult


def _split(nc, psrc, hdst, ldst):
    """psum f32 -> hdst f16 (round) and ldst f16 (residual), both on DVE."""
    nc.vector.tensor_copy(hdst, psrc)
    nc.vector.tensor_sub(ldst, psrc, hdst)


_PHASES = int(os.environ.get("KPHASES", "99"))


def build_nc():
    nc = bacc.Bacc("TRN2", target_bir_lowering=False, debug=False)

    xh_d = nc.declare_dram_parameter("xh", [NT, D], F16, isOutput=False)
    xth_d = nc.declare_dram_parameter("xth", [D, NT], F16, isOutput=False)
    xtl_d = nc.declare_dram_parameter("xtl", [D, NT], F16, isOutput=False)
    xlth_d = nc.declare_dram_parameter("xlth", [D, NL], F16, isOutput=False)
    xltl_d = nc.declare_dram_parameter("xltl", [D, NL], F16, isOutput=False)
    wqh_d = nc.declare_dram_parameter("wqh", [D, D], F16, isOutput=False)
    wql_d = nc.declare_dram_parameter("wql", [D, D], F16, isOutput=False)
    wkth_d = nc.declare_dram_parameter("wkth", [D, D], F16, isOutput=False)
    wktl_d = nc.declare_dram_parameter("wktl", [D, D], F16, isOutput=False)
    out_d = nc.declare_dram_parameter("out_local", [NL, D], F32, isOutput=True)

    with tile.TileContext(nc) as tc:
        with (
            tc.tile_pool(name="persist", bufs=1) as persist,
            tc.tile_pool(name="stats", bufs=1) as stats,
        ):
            ident16 = persist.tile([128, 128], F16, tag="id16", name="id16")
            masks.make_identity(nc, ident16[:])

            # chunk max / exp-pass bias / exp sums, one [128, JC] per m-tile
            pmax = [stats.tile([128, JC], F32, tag=f"pmax{m}", name=f"pmax{m}") for m in range(MT)]
            bsub = [stats.tile([128, JC], F32, tag=f"bsub{m}", name=f"bsub{m}") for m in range(MT)]
            bexp = [stats.tile([128, JC], F32, tag=f"bexp{m}", name=f"bexp{m}") for m in range(MT)]
            esum = [stats.tile([128, JC], F32, tag=f"esum{m}", name=f"esum{m}") for m in range(MT)]
            recip = stats.tile([128, MT], F32, tag="recip", name="recip")

            cth = persist.tile([128, ET, NL], F16, tag="cth", name="cth")
            ctl = persist.tile([128, ET, NL], F16, tag="ctl", name="ctl")
            s16 = [persist.tile([128, JC, 512], F16, tag=f"s16_{m}", name=f"s16_{m}")
                   for m in range(MT)]
            xh_sb = persist.tile([128, NT // 128, D], F16, tag="xhsb", name="xhsb")

            # ---------------- P1 + P2: Q^T then C^T ----------------
            with tc.tile_pool(name="qt", bufs=1) as qtp:
                qth = qtp.tile([128, ET, NL], F16, tag="qth", name="qth")
                qtl = qtp.tile([128, ET, NL], F16, tag="qtl", name="qtl")

                with tc.tile_pool(name="psA", bufs=4, space=bass.MemorySpace.PSUM) as psA:
                    with tc.tile_pool(name="w1", bufs=1) as w1:
                        wqh_sb = w1.tile([128, ET, D], F16, tag="wqh", name="wqh")
                        wql_sb = w1.tile([128, ET, D], F16, tag="wql", name="wql")
                        xlth_sb = w1.tile([128, ET, NL], F16, tag="xlth", name="xlth")
                        xltl_sb = w1.tile([128, ET, NL], F16, tag="xltl", name="xltl")
                        # per-es pieces so the first matmuls can start early;
                        # wqh first: the first Ldweights needs it
                        for es in range(ET):
                            r = slice(es * 128, (es + 1) * 128)
                            nc.sync.dma_start(wqh_sb[:, es, :], wqh_d[r, :])
                            nc.sync.dma_start(xlth_sb[:, es, :], xlth_d[r, :])
                            nc.sync.dma_start(wql_sb[:, es, :], wql_d[r, :])
                            nc.sync.dma_start(xltl_sb[:, es, :], xltl_d[r, :])

                        for dp in range(ET if _PHASES >= 1 else 0):
                            pq = psA.tile([128, NL], F32, tag="acc", name="acc")
                            n_mm = 0
                            for es in range(ET):
                                dcol = slice(dp * 128, (dp + 1) * 128)
                                for lh, rh in ((wqh_sb, xlth_sb), (wqh_sb, xltl_sb),
                                               (wql_sb, xlth_sb)):
                                    nc.tensor.matmul(pq[:], lh[:, es, dcol], rh[:, es, :],
                                                     start=(n_mm == 0), stop=(n_mm == 23))
                                    n_mm += 1
                            _split(nc, pq[:], qth[:, dp, :], qtl[:, dp, :])

                    with tc.tile_pool(name="w2", bufs=1) as w2:
                        wkth_sb = w2.tile([128, ET, D], F16, tag="wkth", name="wkth")
                        wktl_sb = w2.tile([128, ET, D], F16, tag="wktl", name="wktl")
                        for ds in range(ET):
                            r = slice(ds * 128, (ds + 1) * 128)
                            nc.sync.dma_start(wkth_sb[:, ds, :], wkth_d[r, :])
                            nc.sync.dma_start(wktl_sb[:, ds, :], wktl_d[r, :])
                        # prefetch first S chunk while P1/P2 compute
                        x0h = persist.tile([128, ET, 512], F16, tag="x0h", name="x0h")
                        x0l = persist.tile([128, ET, 512], F16, tag="x0l", name="x0l")
                        nc.sync.dma_start(
                            x0h[:], xth_d[:, 0:512].rearrange("(es p) j -> p es j", p=128))
                        nc.sync.dma_start(
                            x0l[:], xtl_d[:, 0:512].rearrange("(es p) j -> p es j", p=128))

                        for ep in range(ET if _PHASES >= 2 else 0):
                            pc = psA.tile([128, NL], F32, tag="acc", name="acc")
                            n_mm = 0
                            for ds in range(ET):
                                ecol = slice(ep * 128, (ep + 1) * 128)
                                for lh, rh in ((wkth_sb, qth), (wkth_sb, qtl),
                                               (wktl_sb, qth)):
                                    nc.tensor.matmul(pc[:], lh[:, ds, ecol], rh[:, ds, :],
                                                     start=(n_mm == 0), stop=(n_mm == 23))
                                    n_mm += 1
                            _split(nc, pc[:], cth[:, ep, :], ctl[:, ep, :])

                    # ------------- P3: S chunks, chunk-max, fp16 store -------------
                    with tc.tile_pool(name="stream", bufs=2) as stream:
                        for jc in range(JC if _PHASES >= 3 else 0):
                            cols = slice(jc * 512, (jc + 1) * 512)
                            if jc == 0:
                                xch, xcl = x0h, x0l
                            else:
                                xch = stream.tile([128, ET, 512], F16, tag="xch", name="xch")
                                xcl = stream.tile([128, ET, 512], F16, tag="xcl", name="xcl")
                                nc.sync.dma_start(
                                    xch[:], xth_d[:, cols].rearrange("(es p) j -> p es j", p=128))
                                nc.sync.dma_start(
                                    xcl[:], xtl_d[:, cols].rearrange("(es p) j -> p es j", p=128))
                            # V rows for this chunk (used in P5), overlapped here
                            nc.sync.dma_start(
                                xh_sb[:, jc * 4:(jc + 1) * 4, :],
                                xh_d[jc * 512:(jc + 1) * 512, :]
                                .rearrange("(jt p) d -> p jt d", p=128))

                            for m in range(MT):
                                ps = psA.tile([128, 512], F32, tag="acc", name="acc")
                                mcol = slice(m * 128, (m + 1) * 128)
                                n_mm = 0
                                for es in range(ET):
                                    for lh, rh in ((cth, xch), (cth, xcl), (ctl, xch)):
                                        nc.tensor.matmul(ps[:], lh[:, es, mcol], rh[:, es, :],
                                                         start=(n_mm == 0), stop=(n_mm == 23))
                                        n_mm += 1
                                nc.vector.reduce_max(pmax[m][:, jc:jc + 1], ps[:], axis=AX)
                                nc.vector.tensor_scalar_mul(
                                    bsub[m][:, jc:jc + 1], pmax[m][:, jc:jc + 1], -SCALE)
                                nc.scalar.activation(s16[m][:, jc, :], ps[:], IDENT,
                                                     bias=bsub[m][:, jc:jc + 1], scale=SCALE)

            # ---------------- P4: global row max -> exp biases ----------------
            for m in range(MT if _PHASES >= 4 else 0):
                rowmax = stats.tile([128, 1], F32, tag=f"rmax{m}", name=f"rmax{m}")
                nc.vector.reduce_max(rowmax[:], pmax[m][:], axis=AX)
                # bexp = (pmax - rowmax) * SCALE
                nc.vector.tensor_scalar(bexp[m][:], pmax[m][:], rowmax[:], SCALE,
                                        op0=SUB, op1=MUL)

            # ---------------- P5: exp, P^T, P@V, scale ----------------
            with (
                tc.tile_pool(name="pvps", bufs=2, space=bass.MemorySpace.PSUM) as pvps,
                tc.tile_pool(name="ptps", bufs=3, space=bass.MemorySpace.PSUM) as ptps,
                tc.tile_pool(name="p5s", bufs=3) as p5s,
            ):
                def p5_stage(m, jc):
                    """exp -> 4 PE transposes -> one DVE copy; returns P^T tiles."""
                    pch = p5s.tile([128, 512], F16, tag="pch", name="pch")
                    nc.scalar.activation(pch[:], s16[m][:, jc, :], EXP,
                                         bias=bexp[m][:, jc:jc + 1], scale=1.0,
                                         accum_out=esum[m][:, jc:jc + 1])
                    ptt = ptps.tile([128, 4, 128], F16, tag="ptt", name="ptt")
                    for js in range(4):
                        nc.tensor.transpose(ptt[:, js, :],
                                            pch[:, js * 128:(js + 1) * 128], ident16[:])
                    pts = p5s.tile([128, 4, 128], F16, tag="pts", name="pts")
                    nc.vector.tensor_copy(pts[:], ptt[:])
                    return pts

                # one global software pipeline over (m, jc): stage k+1 is
                # emitted ahead of chunk k's PV matmuls so the PE never waits
                # on the DVE PSUM->SBUF copy of its lhsT tiles, including
                # across m boundaries
                NS = (MT if _PHASES >= 5 else 0) * JC
                pv = None
                pts_cur = p5_stage(0, 0) if NS else None
                for k in range(NS):
                    m, jc = divmod(k, JC)
                    if jc == 0:
                        pv = pvps.tile([128, 2, 512], F32, tag="pv", name="pv")
                    if k + 1 < NS:
                        m1, jc1 = divmod(k + 1, JC)
                        pts_nxt = p5_stage(m1, jc1)
                    else:
                        pts_nxt = None
                    for js in range(4):
                        jt = jc * 4 + js
                        for n in range(2):
                            nc.tensor.matmul(
                                pv[:, n, :], pts_cur[:, js, :],
                                xh_sb[:, jt, n * 512:(n + 1) * 512],
                                start=(jt == 0), stop=(jt == NT // 128 - 1))
                    pts_cur = pts_nxt
                    if jc == JC - 1:
                        rs = stats.tile([128, 1], F32, tag=f"rs{m}", name=f"rs{m}")
                        nc.vector.reduce_sum(rs[:], esum[m][:], axis=AX)
                        nc.vector.reciprocal(recip[:, m:m + 1], rs[:])
                        for n in range(2):
                            osb = p5s.tile([128, 512], F32, tag="osb", name="osb")
                            nc.vector.tensor_scalar_mul(osb[:], pv[:, n, :], recip[:, m:m + 1])
                            nc.sync.dma_start(
                                out_d[m * 128:(m + 1) * 128, n * 512:(n + 1) * 512], osb[:])

    nc.compile()
    return nc


_NC_CACHE = None


def _split16(a):
    h = a.astype(np.float16)
    l = (a - h.astype(np.float32)).astype(np.float16)
    return h, l


def kernel(inputs, rotation_params, entangle_params):
    global _NC_CACHE
    if _NC_CACHE is None:
        _NC_CACHE = build_nc()
    nc = _NC_CACHE

    x = np.ascontiguousarray(np.asarray(inputs, np.float32))
    wq = np.ascontiguousarray(np.asarray(rotation_params, np.float32))
    wkt = np.ascontiguousarray(np.asarray(entangle_params, np.float32).T)
    xt = np.ascontiguousarray(x.T)

    xh, _ = _split16(x)
    xth, xtl = _split16(xt)
    wqh, wql = _split16(wq)
    wkth, wktl = _split16(wkt)

    in_maps = []
    for c in range(NC):
        cols = slice(c * NL, (c + 1) * NL)
        in_maps.append({
            "xh": xh, "xth": xth, "xtl": xtl,
            "xlth": np.ascontiguousarray(xth[:, cols]),
            "xltl": np.ascontiguousarray(xtl[:, cols]),
            "wqh": wqh, "wql": wql, "wkth": wkth, "wktl": wktl,
        })
    r = run_bass_kernel_spmd(nc, in_maps, list(range(NC)))
    return np.concatenate([r.results[c]["out_local"] for c in range(NC)], axis=0)


# revision 16
# speedup vs baseline: 1.0659x; 1.0030x over previous
"""ClassicalSelfAttention TRN2 kernel — 8-core SPMD, sequence-parallel.

out = softmax((X Wq)(X Wk)^T / sqrt(d)) @ X,  X:[4096,1024] f32, W:[1024,1024].

Per core (rows sharded 8x512), using S_l = ((Xl Wq) Wk^T) X^T so no
replicated projection work:
  Q^T = Wq^T Xl^T        fp16x2 (hh+hl+lh), lhsT = Wq natural layout
  C^T = Wk   Q^T         fp16x2, lhsT = Wk^T (host-transposed)
  S   = C    X^T         fp16x2, rhs = X^T (host-transposed + host-split)
  P   = softmax(S/32)    2-pass; S stored fp16 as (S - chunkmax)*scale
  out = (P @ X) * 1/rowsum   single-pass fp16 (P^T via PE transpose)

Host prep inside kernel(): transpose + fp16 hi/lo split of X / X^T / Wq /
Wk^T (layout-only work; all FLOPs of the computation run on device).
Logits need ~fp32 precision (top-2 gaps down to 3e-3 post-scale; bf16 or
f32r single-pass flips argmax rows), hence fp16x2 for the whole S chain.
P/V tolerate fp16 single-pass. P^T is consumed tile-by-tile straight out
of PSUM copies, so nothing spills to DRAM.

Schedule notes (from TimelineSim traces):
- w1 and w2 share a scope so the Wk^T loads don't WAR-wait on P1's reads
  of the aliased w1 bytes; first S chunk is prefetched the same way.
- P5 runs one global software pipeline over (m, jc): stage k+1 (exp +
  PE transposes + DVE copy) is emitted ahead of chunk k's PV matmuls.
"""
import os
import numpy as np
import concourse.bass as bass
import concourse.bacc as bacc
import concourse.mybir as mybir
import concourse.tile as tile
from concourse import masks
from concourse.bass_utils import run_bass_kernel_spmd

F32 = mybir.dt.float32
F16 = mybir.dt.float16

D = 1024          # embed dim
NT = 4096         # tokens
NC = 8            # cores
NL = NT // NC     # 512 local rows
ET = D // 128     # 8 embed tiles
JC = NT // 512    # 8 j-chunks
MT = NL // 128    # 4 m-tiles
SCALE = float(1.0 / np.sqrt(np.float32(D)))

EXP = mybir.ActivationFunctionType.Exp
IDENT = mybir.ActivationFunctionType.Identity
AX = mybir.AxisListType.X
SUB = mybir.AluOpType.subtract
MUL = mybir.AluOpType.mult

_PHASES = int(os.environ.get("KPHASES", "99"))


def _split(nc, psrc, hdst, ldst):
    """psum f32 -> hdst f16 (round) and ldst f16 (residual), both on DVE."""
    nc.vector.tensor_copy(hdst, psrc)
    nc.vector.tensor_sub(ldst, psrc, hdst)


def build_nc():
    nc = bacc.Bacc("TRN2", target_bir_lowering=False, debug=False)

    xh_d = nc.declare_dram_parameter("xh", [NT, D], F16, isOutput=False)
    xth_d = nc.declare_dram_parameter("xth", [D, NT], F16, isOutput=False)
    xtl_d = nc.declare_dram_parameter("xtl", [D, NT], F16, isOutput=False)
    xlth_d = nc.declare_dram_parameter("xlth", [D, NL], F16, isOutput=False)
    xltl_d = nc.declare_dram_parameter("xltl", [D, NL], F16, isOutput=False)
    wqh_d = nc.declare_dram_parameter("wqh", [D, D], F16, isOutput=False)
    wql_d = nc.declare_dram_parameter("wql", [D, D], F16, isOutput=False)
    wkth_d = nc.declare_dram_parameter("wkth", [D, D], F16, isOutput=False)
    wktl_d = nc.declare_dram_parameter("wktl", [D, D], F16, isOutput=False)
    out_d = nc.declare_dram_parameter("out_local", [NL, D], F32, isOutput=True)

    with tile.TileContext(nc) as tc:
        with (
            tc.tile_pool(name="persist", bufs=1) as persist,
            tc.tile_pool(name="stats", bufs=1) as stats,
        ):
            ident16 = persist.tile([128, 128], F16, tag="id16", name="id16")
            masks.make_identity(nc, ident16[:])

            # chunk max / exp-pass bias / exp sums, one [128, JC] per m-tile
            pmax = [stats.tile([128, JC], F32, tag=f"pmax{m}", name=f"pmax{m}") for m in range(MT)]
            bsub = [stats.tile([128, JC], F32, tag=f"bsub{m}", name=f"bsub{m}") for m in range(MT)]
            bexp = [stats.tile([128, JC], F32, tag=f"bexp{m}", name=f"bexp{m}") for m in range(MT)]
            esum = [stats.tile([128, JC], F32, tag=f"esum{m}", name=f"esum{m}") for m in range(MT)]
            recip = stats.tile([128, MT], F32, tag="recip", name="recip")

            cth = persist.tile([128, ET, NL], F16, tag="cth", name="cth")
            ctl = persist.tile([128, ET, NL], F16, tag="ctl", name="ctl")
            s16 = [persist.tile([128, JC, 512], F16, tag=f"s16_{m}", name=f"s16_{m}")
                   for m in range(MT)]
            x0h = persist.tile([128, ET, 512], F16, tag="x0h", name="x0h")
            x0l = persist.tile([128, ET, 512], F16, tag="x0l", name="x0l")

            # ---------------- P1 + P2: Q^T then C^T ----------------
            with tc.tile_pool(name="qt", bufs=1) as qtp:
                qth = qtp.tile([128, ET, NL], F16, tag="qth", name="qth")
                qtl = qtp.tile([128, ET, NL], F16, tag="qtl", name="qtl")

                with (
                    tc.tile_pool(name="psA", bufs=4, space=bass.MemorySpace.PSUM) as psA,
                    tc.tile_pool(name="w1", bufs=1) as w1,
                    tc.tile_pool(name="w2", bufs=1) as w2,
                ):
                    wqh_sb = w1.tile([128, ET, D], F16, tag="wqh", name="wqh")
                    wql_sb = w1.tile([128, ET, D], F16, tag="wql", name="wql")
                    xlth_sb = w1.tile([128, ET, NL], F16, tag="xlth", name="xlth")
                    xltl_sb = w1.tile([128, ET, NL], F16, tag="xltl", name="xltl")
                    wkth_sb = w2.tile([128, ET, D], F16, tag="wkth", name="wkth")
                    wktl_sb = w2.tile([128, ET, D], F16, tag="wktl", name="wktl")

                    # per-es pieces so the first matmuls can start early;
                    # wqh first: the first Ldweights needs it
                    for es in range(ET):
                        r = slice(es * 128, (es + 1) * 128)
                        nc.sync.dma_start(wqh_sb[:, es, :], wqh_d[r, :])
                        nc.sync.dma_start(xlth_sb[:, es, :], xlth_d[r, :])
                        nc.sync.dma_start(wql_sb[:, es, :], wql_d[r, :])
                        nc.sync.dma_start(xltl_sb[:, es, :], xltl_d[r, :])
                    # Wk^T and the first S chunk stream in during P1 compute
                    for ds in range(ET):
                        r = slice(ds * 128, (ds + 1) * 128)
                        nc.sync.dma_start(wkth_sb[:, ds, :], wkth_d[r, :])
                        nc.sync.dma_start(wktl_sb[:, ds, :], wktl_d[r, :])
                    nc.sync.dma_start(
                        x0h[:], xth_d[:, 0:512].rearrange("(es p) j -> p es j", p=128))
                    nc.sync.dma_start(
                        x0l[:], xtl_d[:, 0:512].rearrange("(es p) j -> p es j", p=128))

                    for dp in range(ET if _PHASES >= 1 else 0):
                        pq = psA.tile([128, NL], F32, tag="acc", name="acc")
                        n_mm = 0
                        for es in range(ET):
                            dcol = slice(dp * 128, (dp + 1) * 128)
                            for lh, rh in ((wqh_sb, xlth_sb), (wqh_sb, xltl_sb),
                                           (wql_sb, xlth_sb)):
                                nc.tensor.matmul(pq[:], lh[:, es, dcol], rh[:, es, :],
                                                 start=(n_mm == 0), stop=(n_mm == 23))
                                n_mm += 1
                        _split(nc, pq[:], qth[:, dp, :], qtl[:, dp, :])

                    for ep in range(ET if _PHASES >= 2 else 0):
                        pc = psA.tile([128, NL], F32, tag="acc", name="acc")
                        n_mm = 0
                        for ds in range(ET):
                            ecol = slice(ep * 128, (ep + 1) * 128)
                            for lh, rh in ((wkth_sb, qth), (wkth_sb, qtl),
                                           (wktl_sb, qth)):
                                nc.tensor.matmul(pc[:], lh[:, ds, ecol], rh[:, ds, :],
                                                 start=(n_mm == 0), stop=(n_mm == 23))
                                n_mm += 1
                        _split(nc, pc[:], cth[:, ep, :], ctl[:, ep, :])

            # ---------------- P3: S chunks, chunk-max, fp16 store ----------------
            with tc.tile_pool(name="xhp", bufs=1) as xhp:
                xh_sb = xhp.tile([128, NT // 128, D], F16, tag="xhsb", name="xhsb")
                with (
                    tc.tile_pool(name="psC", bufs=4, space=bass.MemorySpace.PSUM) as psC,
                    tc.tile_pool(name="stream", bufs=2) as stream,
                ):
                    for jc in range(JC if _PHASES >= 3 else 0):
                        cols = slice(jc * 512, (jc + 1) * 512)
                        if jc == 0:
                            xch, xcl = x0h, x0l
                        else:
                            xch = stream.tile([128, ET, 512], F16, tag="xch", name="xch")
                            xcl = stream.tile([128, ET, 512], F16, tag="xcl", name="xcl")
                            nc.sync.dma_start(
                                xch[:], xth_d[:, cols].rearrange("(es p) j -> p es j", p=128))
                            nc.sync.dma_start(
                                xcl[:], xtl_d[:, cols].rearrange("(es p) j -> p es j", p=128))
                        # V rows for this chunk (used in P5), overlapped here
                        nc.sync.dma_start(
                            xh_sb[:, jc * 4:(jc + 1) * 4, :],
                            xh_d[jc * 512:(jc + 1) * 512, :]
                            .rearrange("(jt p) d -> p jt d", p=128))

                        for m in range(MT):
                            ps = psC.tile([128, 512], F32, tag="acc", name="acc")
                            mcol = slice(m * 128, (m + 1) * 128)
                            n_mm = 0
                            for es in range(ET):
                                for lh, rh in ((cth, xch), (cth, xcl), (ctl, xch)):
                                    nc.tensor.matmul(ps[:], lh[:, es, mcol], rh[:, es, :],
                                                     start=(n_mm == 0), stop=(n_mm == 23))
                                    n_mm += 1
                            nc.vector.reduce_max(pmax[m][:, jc:jc + 1], ps[:], axis=AX)
                            nc.vector.tensor_scalar_mul(
                                bsub[m][:, jc:jc + 1], pmax[m][:, jc:jc + 1], -SCALE)
                            nc.scalar.activation(s16[m][:, jc, :], ps[:], IDENT,
                                                 bias=bsub[m][:, jc:jc + 1], scale=SCALE)

                # ---------------- P4: global row max -> exp biases ----------------
                for m in range(MT if _PHASES >= 4 else 0):
                    rowmax = stats.tile([128, 1], F32, tag=f"rmax{m}", name=f"rmax{m}")
                    nc.vector.reduce_max(rowmax[:], pmax[m][:], axis=AX)
                    # bexp = (pmax - rowmax) * SCALE
                    nc.vector.tensor_scalar(bexp[m][:], pmax[m][:], rowmax[:], SCALE,
                                            op0=SUB, op1=MUL)

                # ---------------- P5: exp, P^T, P@V, scale ----------------
                with (
                    tc.tile_pool(name="pvps", bufs=2, space=bass.MemorySpace.PSUM) as pvps,
                    tc.tile_pool(name="ptps", bufs=3, space=bass.MemorySpace.PSUM) as ptps,
                    tc.tile_pool(name="p5s", bufs=3) as p5s,
                ):
                    def p5_stage(m, jc):
                        """exp -> 4 PE transposes -> one DVE copy; returns P^T tiles."""
                        pch = p5s.tile([128, 512], F16, tag="pch", name="pch")
                        nc.scalar.activation(pch[:], s16[m][:, jc, :], EXP,
                                             bias=bexp[m][:, jc:jc + 1], scale=1.0,
                                             accum_out=esum[m][:, jc:jc + 1])
                        ptt = ptps.tile([128, 4, 128], F16, tag="ptt", name="ptt")
                        for js in range(4):
                            nc.tensor.transpose(ptt[:, js, :],
                                                pch[:, js * 128:(js + 1) * 128], ident16[:])
                        pts = p5s.tile([128, 4, 128], F16, tag="pts", name="pts")
                        nc.vector.tensor_copy(pts[:], ptt[:])
                        return pts

                    # one global software pipeline over (m, jc): stage k+1 is
                    # emitted ahead of chunk k's PV matmuls so the PE never
                    # waits on the DVE PSUM->SBUF copy of its lhsT tiles,
                    # including across m boundaries
                    NS = (MT if _PHASES >= 5 else 0) * JC
                    pv = None
                    pts_cur = p5_stage(0, 0) if NS else None
                    for k in range(NS):
                        m, jc = divmod(k, JC)
                        if jc == 0:
                            pv = pvps.tile([128, 2, 512], F32, tag="pv", name="pv")
                        if k + 1 < NS:
                            m1, jc1 = divmod(k + 1, JC)
                            pts_nxt = p5_stage(m1, jc1)
                        else:
                            pts_nxt = None
                        for js in range(4):
                            jt = jc * 4 + js
                            for n in range(2):
                                nc.tensor.matmul(
                                    pv[:, n, :], pts_cur[:, js, :],
                                    xh_sb[:, jt, n * 512:(n + 1) * 512],
                                    start=(jt == 0), stop=(jt == NT // 128 - 1))
                        pts_cur = pts_nxt
                        if jc == JC - 1:
                            rs = stats.tile([128, 1], F32, tag=f"rs{m}", name=f"rs{m}")
                            nc.vector.reduce_sum(rs[:], esum[m][:], axis=AX)
                            nc.vector.reciprocal(recip[:, m:m + 1], rs[:])
                            for n in range(2):
                                osb = p5s.tile([128, 512], F32, tag="osb", name="osb")
                                nc.vector.tensor_scalar_mul(osb[:], pv[:, n, :],
                                                            recip[:, m:m + 1])
                                nc.sync.dma_start(
                                    out_d[m * 128:(m + 1) * 128,
                                          n * 512:(n + 1) * 512], osb[:])

    nc.compile()
    return nc


_NC_CACHE = None


def _split16(a):
    h = a.astype(np.float16)
    l = (a - h.astype(np.float32)).astype(np.float16)
    return h, l


def kernel(inputs, rotation_params, entangle_params):
    global _NC_CACHE
    if _NC_CACHE is None:
        _NC_CACHE = build_nc()
    nc = _NC_CACHE

    x = np.ascontiguousarray(np.asarray(inputs, np.float32))
    wq = np.ascontiguousarray(np.asarray(rotation_params, np.float32))
    wkt = np.ascontiguousarray(np.asarray(entangle_params, np.float32).T)
    xt = np.ascontiguousarray(x.T)

    xh, _ = _split16(x)
    xth, xtl = _split16(xt)
    wqh, wql = _split16(wq)
    wkth, wktl = _split16(wkt)

    in_maps = []
    for c in range(NC):
        cols = slice(c * NL, (c + 1) * NL)
        in_maps.append({
            "xh": xh, "xth": xth, "xtl": xtl,
            "xlth": np.ascontiguousarray(xth[:, cols]),
            "xltl": np.ascontiguousarray(xtl[:, cols]),
            "wqh": wqh, "wql": wql, "wkth": wkth, "wktl": wktl,
        })
    r = run_bass_kernel_spmd(nc, in_maps, list(range(NC)))
    return np.concatenate([r.results[c]["out_local"] for c in range(NC)], axis=0)


# revision 17
# speedup vs baseline: 1.0669x; 1.0009x over previous
"""ClassicalSelfAttention TRN2 kernel — 8-core SPMD, sequence-parallel.

out = softmax((X Wq)(X Wk)^T / sqrt(d)) @ X,  X:[4096,1024] f32, W:[1024,1024].

Per core (rows sharded 8x512), using S_l = ((Xl Wq) Wk^T) X^T so no
replicated projection work:
  Q^T = Wq^T Xl^T        fp16x2 (hh+hl+lh), lhsT = Wq natural layout
  C^T = Wk   Q^T         fp16x2, lhsT = Wk^T (host-transposed)
  S   = C    X^T         fp16x2, rhs = X^T (host-transposed + host-split)
  P   = softmax(S/32)    2-pass; S stored fp16 as (S - chunkmax)*scale
  out = (P @ X) * 1/rowsum   single-pass fp16 (P^T via PE transpose)

Host prep inside kernel(): transpose + fp16 hi/lo split of X / X^T / Wq /
Wk^T (layout-only work; all FLOPs of the computation run on device).
Logits need ~fp32 precision (top-2 gaps down to 3e-3 post-scale; bf16 or
f32r single-pass flips argmax rows), hence fp16x2 for the whole S chain.
P/V tolerate fp16 single-pass. P^T is consumed tile-by-tile straight out
of PSUM copies, so nothing spills to DRAM.

Schedule notes (from TimelineSim traces):
- w1 and w2 share a scope so the Wk^T loads don't WAR-wait on P1's reads
  of the aliased w1 bytes; first S chunk is prefetched the same way.
- P5 runs one global software pipeline over (m, jc): stage k+1 (exp +
  PE transposes + DVE copy) is emitted ahead of chunk k's PV matmuls.
"""
import os
import numpy as np
import concourse.bass as bass
import concourse.bacc as bacc
import concourse.mybir as mybir
import concourse.tile as tile
from concourse import masks
from concourse.bass_utils import run_bass_kernel_spmd

F32 = mybir.dt.float32
F16 = mybir.dt.float16

D = 1024          # embed dim
NT = 4096         # tokens
NC = 8            # cores
NL = NT // NC     # 512 local rows
ET = D // 128     # 8 embed tiles
JC = NT // 512    # 8 j-chunks
MT = NL // 128    # 4 m-tiles
SCALE = float(1.0 / np.sqrt(np.float32(D)))

EXP = mybir.ActivationFunctionType.Exp
IDENT = mybir.ActivationFunctionType.Identity
AX = mybir.AxisListType.X
SUB = mybir.AluOpType.subtract
MUL = mybir.AluOpType.mult

_PHASES = int(os.environ.get("KPHASES", "99"))


def _split(nc, psrc, hdst, ldst):
    """psum f32 -> hdst f16 (round) and ldst f16 (residual), both on DVE."""
    nc.vector.tensor_copy(hdst, psrc)
    nc.vector.tensor_sub(ldst, psrc, hdst)


def build_nc():
    nc = bacc.Bacc("TRN2", target_bir_lowering=False, debug=False)

    xh_d = nc.declare_dram_parameter("xh", [NT, D], F16, isOutput=False)
    xth_d = nc.declare_dram_parameter("xth", [D, NT], F16, isOutput=False)
    xtl_d = nc.declare_dram_parameter("xtl", [D, NT], F16, isOutput=False)
    xlth_d = nc.declare_dram_parameter("xlth", [D, NL], F16, isOutput=False)
    xltl_d = nc.declare_dram_parameter("xltl", [D, NL], F16, isOutput=False)
    wqh_d = nc.declare_dram_parameter("wqh", [D, D], F16, isOutput=False)
    wql_d = nc.declare_dram_parameter("wql", [D, D], F16, isOutput=False)
    wkth_d = nc.declare_dram_parameter("wkth", [D, D], F16, isOutput=False)
    wktl_d = nc.declare_dram_parameter("wktl", [D, D], F16, isOutput=False)
    out_d = nc.declare_dram_parameter("out_local", [NL, D], F32, isOutput=True)

    with tile.TileContext(nc) as tc:
        with (
            tc.tile_pool(name="persist", bufs=1) as persist,
            tc.tile_pool(name="stats", bufs=1) as stats,
        ):
            ident16 = persist.tile([128, 128], F16, tag="id16", name="id16")
            masks.make_identity(nc, ident16[:])

            # chunk max / exp-pass bias / exp sums, one [128, JC] per m-tile
            pmax = [stats.tile([128, JC], F32, tag=f"pmax{m}", name=f"pmax{m}") for m in range(MT)]
            bsub = [stats.tile([128, JC], F32, tag=f"bsub{m}", name=f"bsub{m}") for m in range(MT)]
            bexp = [stats.tile([128, JC], F32, tag=f"bexp{m}", name=f"bexp{m}") for m in range(MT)]
            esum = [stats.tile([128, JC], F32, tag=f"esum{m}", name=f"esum{m}") for m in range(MT)]
            recip = stats.tile([128, MT], F32, tag="recip", name="recip")

            cth = persist.tile([128, ET, NL], F16, tag="cth", name="cth")
            ctl = persist.tile([128, ET, NL], F16, tag="ctl", name="ctl")
            s16 = [persist.tile([128, JC, 512], F16, tag=f"s16_{m}", name=f"s16_{m}")
                   for m in range(MT)]
            x0h = persist.tile([128, ET, 512], F16, tag="x0h", name="x0h")
            x0l = persist.tile([128, ET, 512], F16, tag="x0l", name="x0l")

            # ---------------- P1 + P2: Q^T then C^T ----------------
            with tc.tile_pool(name="qt", bufs=1) as qtp:
                qth = qtp.tile([128, ET, NL], F16, tag="qth", name="qth")
                qtl = qtp.tile([128, ET, NL], F16, tag="qtl", name="qtl")

                with (
                    tc.tile_pool(name="psA", bufs=4, space=bass.MemorySpace.PSUM) as psA,
                    tc.tile_pool(name="w1", bufs=1) as w1,
                    tc.tile_pool(name="w2", bufs=1) as w2,
                ):
                    wqh_sb = w1.tile([128, ET, D], F16, tag="wqh", name="wqh")
                    wql_sb = w1.tile([128, ET, D], F16, tag="wql", name="wql")
                    xlth_sb = w1.tile([128, ET, NL], F16, tag="xlth", name="xlth")
                    xltl_sb = w1.tile([128, ET, NL], F16, tag="xltl", name="xltl")
                    wkth_sb = w2.tile([128, ET, D], F16, tag="wkth", name="wkth")
                    wktl_sb = w2.tile([128, ET, D], F16, tag="wktl", name="wktl")

                    # per-es pieces so the first matmuls can start early;
                    # wqh first: the first Ldweights needs it
                    for es in range(ET):
                        r = slice(es * 128, (es + 1) * 128)
                        nc.sync.dma_start(wqh_sb[:, es, :], wqh_d[r, :])
                        nc.sync.dma_start(xlth_sb[:, es, :], xlth_d[r, :])
                        nc.sync.dma_start(wql_sb[:, es, :], wql_d[r, :])
                        nc.sync.dma_start(xltl_sb[:, es, :], xltl_d[r, :])
                    # Wk^T and the first S chunk stream in during P1 compute
                    for ds in range(ET):
                        r = slice(ds * 128, (ds + 1) * 128)
                        nc.sync.dma_start(wkth_sb[:, ds, :], wkth_d[r, :])
                        nc.sync.dma_start(wktl_sb[:, ds, :], wktl_d[r, :])
                    nc.sync.dma_start(
                        x0h[:], xth_d[:, 0:512].rearrange("(es p) j -> p es j", p=128))
                    nc.sync.dma_start(
                        x0l[:], xtl_d[:, 0:512].rearrange("(es p) j -> p es j", p=128))

                    for dp in range(ET if _PHASES >= 1 else 0):
                        pq = psA.tile([128, NL], F32, tag="acc", name="acc")
                        n_mm = 0
                        for es in range(ET):
                            dcol = slice(dp * 128, (dp + 1) * 128)
                            for lh, rh in ((wqh_sb, xlth_sb), (wqh_sb, xltl_sb),
                                           (wql_sb, xlth_sb)):
                                nc.tensor.matmul(pq[:], lh[:, es, dcol], rh[:, es, :],
                                                 start=(n_mm == 0), stop=(n_mm == 23))
                                n_mm += 1
                        _split(nc, pq[:], qth[:, dp, :], qtl[:, dp, :])

                    for ep in range(ET if _PHASES >= 2 else 0):
                        pc = psA.tile([128, NL], F32, tag="acc", name="acc")
                        n_mm = 0
                        for ds in range(ET):
                            ecol = slice(ep * 128, (ep + 1) * 128)
                            for lh, rh in ((wkth_sb, qth), (wkth_sb, qtl),
                                           (wktl_sb, qth)):
                                nc.tensor.matmul(pc[:], lh[:, ds, ecol], rh[:, ds, :],
                                                 start=(n_mm == 0), stop=(n_mm == 23))
                                n_mm += 1
                        _split(nc, pc[:], cth[:, ep, :], ctl[:, ep, :])

            # ---------------- P3: S chunks, chunk-max, fp16 store ----------------
            with (
                tc.tile_pool(name="xhp", bufs=1) as xhp,
                tc.tile_pool(name="ptps", bufs=3, space=bass.MemorySpace.PSUM) as ptps,
                tc.tile_pool(name="p5s", bufs=3) as p5s,
            ):
                xh_sb = xhp.tile([128, NT // 128, D], F16, tag="xhsb", name="xhsb")

                def p4_stats(m):
                    """global row max of m -> exp-pass bias row."""
                    rowmax = stats.tile([128, 1], F32, tag=f"rmax{m}", name=f"rmax{m}")
                    nc.vector.reduce_max(rowmax[:], pmax[m][:], axis=AX)
                    # bexp = (pmax - rowmax) * SCALE
                    nc.vector.tensor_scalar(bexp[m][:], pmax[m][:], rowmax[:], SCALE,
                                            op0=SUB, op1=MUL)

                def p5_stage(m, jc):
                    """exp -> 4 PE transposes -> one DVE copy; returns P^T tiles."""
                    pch = p5s.tile([128, 512], F16, tag="pch", name="pch")
                    nc.scalar.activation(pch[:], s16[m][:, jc, :], EXP,
                                         bias=bexp[m][:, jc:jc + 1], scale=1.0,
                                         accum_out=esum[m][:, jc:jc + 1])
                    ptt = ptps.tile([128, 4, 128], F16, tag="ptt", name="ptt")
                    for js in range(4):
                        nc.tensor.transpose(ptt[:, js, :],
                                            pch[:, js * 128:(js + 1) * 128], ident16[:])
                    pts = p5s.tile([128, 4, 128], F16, tag="pts", name="pts")
                    nc.vector.tensor_copy(pts[:], ptt[:])
                    return pts

                pts0 = None
                with (
                    tc.tile_pool(name="psC", bufs=4, space=bass.MemorySpace.PSUM) as psC,
                    tc.tile_pool(name="stream", bufs=2) as stream,
                ):
                    for jc in range(JC if _PHASES >= 3 else 0):
                        cols = slice(jc * 512, (jc + 1) * 512)
                        if jc == 0:
                            xch, xcl = x0h, x0l
                        else:
                            xch = stream.tile([128, ET, 512], F16, tag="xch", name="xch")
                            xcl = stream.tile([128, ET, 512], F16, tag="xcl", name="xcl")
                            nc.sync.dma_start(
                                xch[:], xth_d[:, cols].rearrange("(es p) j -> p es j", p=128))
                            nc.sync.dma_start(
                                xcl[:], xtl_d[:, cols].rearrange("(es p) j -> p es j", p=128))
                        # V rows for this chunk (used in P5), overlapped here
                        nc.sync.dma_start(
                            xh_sb[:, jc * 4:(jc + 1) * 4, :],
                            xh_d[jc * 512:(jc + 1) * 512, :]
                            .rearrange("(jt p) d -> p jt d", p=128))

                        for m in range(MT):
                            ps = psC.tile([128, 512], F32, tag="acc", name="acc")
                            mcol = slice(m * 128, (m + 1) * 128)
                            n_mm = 0
                            for es in range(ET):
                                for lh, rh in ((cth, xch), (cth, xcl), (ctl, xch)):
                                    nc.tensor.matmul(ps[:], lh[:, es, mcol], rh[:, es, :],
                                                     start=(n_mm == 0), stop=(n_mm == 23))
                                    n_mm += 1
                            nc.vector.reduce_max(pmax[m][:, jc:jc + 1], ps[:], axis=AX)
                            nc.vector.tensor_scalar_mul(
                                bsub[m][:, jc:jc + 1], pmax[m][:, jc:jc + 1], -SCALE)
                            nc.scalar.activation(s16[m][:, jc, :], ps[:], IDENT,
                                                 bias=bsub[m][:, jc:jc + 1], scale=SCALE)
                            if jc == JC - 1 and _PHASES >= 4:
                                # m's chunk stats are complete: fold the P4
                                # bias prep (and the first P5 stage) into the
                                # tail of P3 so P5 starts without a stall
                                p4_stats(m)
                                if m == 0 and _PHASES >= 5:
                                    pts0 = p5_stage(0, 0)

                # ---------------- P5: exp, P^T, P@V, scale ----------------
                with tc.tile_pool(name="pvps", bufs=2,
                                  space=bass.MemorySpace.PSUM) as pvps:
                    # one global software pipeline over (m, jc): stage k+1 is
                    # emitted ahead of chunk k's PV matmuls so the PE never
                    # waits on the DVE PSUM->SBUF copy of its lhsT tiles,
                    # including across m boundaries
                    NS = (MT if _PHASES >= 5 else 0) * JC
                    pv = None
                    pts_cur = pts0
                    for k in range(NS):
                        m, jc = divmod(k, JC)
                        if jc == 0:
                            pv = pvps.tile([128, 2, 512], F32, tag="pv", name="pv")
                        if k + 1 < NS:
                            m1, jc1 = divmod(k + 1, JC)
                            pts_nxt = p5_stage(m1, jc1)
                        else:
                            pts_nxt = None
                        for js in range(4):
                            jt = jc * 4 + js
                            for n in range(2):
                                nc.tensor.matmul(
                                    pv[:, n, :], pts_cur[:, js, :],
                                    xh_sb[:, jt, n * 512:(n + 1) * 512],
                                    start=(jt == 0), stop=(jt == NT // 128 - 1))
                        pts_cur = pts_nxt
                        if jc == JC - 1:
                            rs = stats.tile([128, 1], F32, tag=f"rs{m}", name=f"rs{m}")
                            nc.vector.reduce_sum(rs[:], esum[m][:], axis=AX)
                            nc.vector.reciprocal(recip[:, m:m + 1], rs[:])
                            for n in range(2):
                                osb = p5s.tile([128, 512], F32, tag="osb", name="osb")
                                nc.vector.tensor_scalar_mul(osb[:], pv[:, n, :],
                                                            recip[:, m:m + 1])
                                nc.sync.dma_start(
                                    out_d[m * 128:(m + 1) * 128,
                                          n * 512:(n + 1) * 512], osb[:])

    nc.compile()
    return nc


_NC_CACHE = None


def _split16(a):
    h = a.astype(np.float16)
    l = (a - h.astype(np.float32)).astype(np.float16)
    return h, l


def kernel(inputs, rotation_params, entangle_params):
    global _NC_CACHE
    if _NC_CACHE is None:
        _NC_CACHE = build_nc()
    nc = _NC_CACHE

    x = np.ascontiguousarray(np.asarray(inputs, np.float32))
    wq = np.ascontiguousarray(np.asarray(rotation_params, np.float32))
    wkt = np.ascontiguousarray(np.asarray(entangle_params, np.float32).T)
    xt = np.ascontiguousarray(x.T)

    xh, _ = _split16(x)
    xth, xtl = _split16(xt)
    wqh, wql = _split16(wq)
    wkth, wktl = _split16(wkt)

    in_maps = []
    for c in range(NC):
        cols = slice(c * NL, (c + 1) * NL)
        in_maps.append({
            "xh": xh, "xth": xth, "xtl": xtl,
            "xlth": np.ascontiguousarray(xth[:, cols]),
            "xltl": np.ascontiguousarray(xtl[:, cols]),
            "wqh": wqh, "wql": wql, "wkth": wkth, "wktl": wktl,
        })
    r = run_bass_kernel_spmd(nc, in_maps, list(range(NC)))
    return np.concatenate([r.results[c]["out_local"] for c in range(NC)], axis=0)


# revision 23
# speedup vs baseline: 1.0757x; 1.0083x over previous
"""ClassicalSelfAttention TRN2 kernel — 8-core SPMD, sequence-parallel.

out = softmax((X Wq)(X Wk)^T / sqrt(d)) @ X,  X:[4096,1024] f32, W:[1024,1024].

Per core (rows sharded 8x512), using S_l = ((Xl Wq) Wk^T) X^T so no
replicated projection work:
  Q^T = Wq^T Xl^T        fp16x2 (hh+hl+lh), lhsT = Wq natural layout
  C^T = Wk   Q^T         fp16x2, lhsT = Wk^T (host-transposed)
  S   = C    X^T         fp16x2, rhs = X^T (host-transposed + host-split)
  P   = softmax(S/32)    2-pass; S stored fp16 as (S - chunkmax)*scale
  out = (P @ X) * 1/rowsum   single-pass fp16 (P^T via PE transpose)

Host prep inside kernel(): transpose + fp16 hi/lo split of X / X^T / Wq /
Wk^T (layout-only work; all FLOPs of the computation run on device).
Logits need ~fp32 precision (top-2 gaps down to 3e-3 post-scale; bf16 or
f32r single-pass flips argmax rows), hence fp16x2 for the whole S chain.
P/V tolerate fp16 single-pass. P^T is consumed tile-by-tile straight out
of PSUM copies, so nothing spills to DRAM.

Schedule notes (from TimelineSim traces):
- w1 and w2 share a scope so the Wk^T loads don't WAR-wait on P1's reads
  of the aliased w1 bytes; first S chunk is prefetched the same way.
- P5 runs one global software pipeline over (m, jc): stage k+1 (exp +
  PE transposes + DVE copy) is emitted ahead of chunk k's PV matmuls.
"""
import os
import numpy as np
import concourse.bass as bass
import concourse.bacc as bacc
import concourse.mybir as mybir
import concourse.tile as tile
from concourse import masks
from concourse.bass_utils import run_bass_kernel_spmd

F32 = mybir.dt.float32
F16 = mybir.dt.float16

D = 1024          # embed dim
NT = 4096         # tokens
NC = 8            # cores
NL = NT // NC     # 512 local rows
ET = D // 128     # 8 embed tiles
JC = NT // 512    # 8 j-chunks
MT = NL // 128    # 4 m-tiles
SCALE = float(1.0 / np.sqrt(np.float32(D)))

EXP = mybir.ActivationFunctionType.Exp
IDENT = mybir.ActivationFunctionType.Identity
AX = mybir.AxisListType.X
SUB = mybir.AluOpType.subtract
MUL = mybir.AluOpType.mult

_PHASES = int(os.environ.get("KPHASES", "99"))


def _split(nc, psrc, hdst, ldst):
    """psum f32 -> hdst f16 (round) and ldst f16 (residual), both on DVE."""
    nc.vector.tensor_copy(hdst, psrc)
    nc.vector.tensor_sub(ldst, psrc, hdst)


def build_nc():
    nc = bacc.Bacc("TRN2", target_bir_lowering=False, debug=False)

    xh_d = nc.declare_dram_parameter("xh", [NT, D], F16, isOutput=False)
    xth_d = nc.declare_dram_parameter("xth", [D, NT], F16, isOutput=False)
    xtl_d = nc.declare_dram_parameter("xtl", [D, NT], F16, isOutput=False)
    xlth_d = nc.declare_dram_parameter("xlth", [D, NL], F16, isOutput=False)
    xltl_d = nc.declare_dram_parameter("xltl", [D, NL], F16, isOutput=False)
    wqh_d = nc.declare_dram_parameter("wqh", [D, D], F16, isOutput=False)
    wql_d = nc.declare_dram_parameter("wql", [D, D], F16, isOutput=False)
    wkth_d = nc.declare_dram_parameter("wkth", [D, D], F16, isOutput=False)
    wktl_d = nc.declare_dram_parameter("wktl", [D, D], F16, isOutput=False)
    out_d = nc.declare_dram_parameter("out_local", [NL, D], F32, isOutput=True)

    with tile.TileContext(nc) as tc:
        with (
            tc.tile_pool(name="persist", bufs=1) as persist,
            tc.tile_pool(name="stats", bufs=1) as stats,
            tc.tile_pool(name="ptps", bufs=2, space=bass.MemorySpace.PSUM) as ptps,
            tc.tile_pool(name="p5s", bufs=3) as p5s,
            tc.tile_pool(name="psA", bufs=4, space=bass.MemorySpace.PSUM) as psA,
        ):
            ident16 = persist.tile([128, 128], F16, tag="id16", name="id16")
            masks.make_identity(nc, ident16[:])

            # chunk max / exp-pass bias / exp sums, one [128, JC] per m-tile
            pmax = [stats.tile([128, JC], F32, tag=f"pmax{m}", name=f"pmax{m}") for m in range(MT)]
            bsub = [stats.tile([128, JC], F32, tag=f"bsub{m}", name=f"bsub{m}") for m in range(MT)]
            bexp = [stats.tile([128, JC], F32, tag=f"bexp{m}", name=f"bexp{m}") for m in range(MT)]
            esum = [stats.tile([128, JC], F32, tag=f"esum{m}", name=f"esum{m}") for m in range(MT)]
            recip = stats.tile([128, MT], F32, tag="recip", name="recip")

            cth = [persist.tile([128, NL], F16, tag=f"cth{e}", name=f"cth{e}")
                   for e in range(ET)]
            ctl = [persist.tile([128, NL], F16, tag=f"ctl{e}", name=f"ctl{e}")
                   for e in range(ET)]
            s16 = [persist.tile([128, JC, 512], F16, tag=f"s16_{m}", name=f"s16_{m}")
                   for m in range(MT)]
            x0h = persist.tile([128, ET, 512], F16, tag="x0h", name="x0h")
            x0l = persist.tile([128, ET, 512], F16, tag="x0l", name="x0l")

            # ---------------- P1 + P2: Q^T then C^T ----------------
            with tc.tile_pool(name="qt", bufs=1) as qtp:
                qth = [qtp.tile([128, NL], F16, tag=f"qth{d}", name=f"qth{d}")
                       for d in range(ET)]
                qtl = [qtp.tile([128, NL], F16, tag=f"qtl{d}", name=f"qtl{d}")
                       for d in range(ET)]

                with (
                    tc.tile_pool(name="w1", bufs=1) as w1,
                    tc.tile_pool(name="w2", bufs=1) as w2,
                ):
                    wqh_sb = w1.tile([128, ET, D], F16, tag="wqh", name="wqh")
                    wql_sb = w1.tile([128, ET, D], F16, tag="wql", name="wql")
                    xlth_sb = w1.tile([128, ET, NL], F16, tag="xlth", name="xlth")
                    xltl_sb = w1.tile([128, ET, NL], F16, tag="xltl", name="xltl")
                    wkth_sb = w2.tile([128, ET, D], F16, tag="wkth", name="wkth")
                    wktl_sb = w2.tile([128, ET, D], F16, tag="wktl", name="wktl")

                    # per-es pieces so the first matmuls can start early;
                    # wqh first: the first Ldweights needs it
                    nc.sync.dma_start(wqh_sb[:, 0, 0:512], wqh_d[0:128, 0:512])
                    nc.sync.dma_start(xlth_sb[:, 0, :], xlth_d[0:128, :])
                    nc.sync.dma_start(wqh_sb[:, 0, 512:D], wqh_d[0:128, 512:D])
                    nc.sync.dma_start(wql_sb[:, 0, :], wql_d[0:128, :])
                    nc.sync.dma_start(xltl_sb[:, 0, :], xltl_d[0:128, :])
                    for es in range(1, ET):
                        r = slice(es * 128, (es + 1) * 128)
                        nc.sync.dma_start(wqh_sb[:, es, :], wqh_d[r, :])
                        nc.sync.dma_start(xlth_sb[:, es, :], xlth_d[r, :])
                        nc.sync.dma_start(wql_sb[:, es, :], wql_d[r, :])
                        nc.sync.dma_start(xltl_sb[:, es, :], xltl_d[r, :])
                    # Wk^T and the first S chunk stream in during P1 compute
                    for ds in range(ET):
                        r = slice(ds * 128, (ds + 1) * 128)
                        nc.sync.dma_start(wkth_sb[:, ds, :], wkth_d[r, :])
                        nc.sync.dma_start(wktl_sb[:, ds, :], wktl_d[r, :])
                    nc.sync.dma_start(
                        x0h[:], xth_d[:, 0:512].rearrange("(es p) j -> p es j", p=128))
                    nc.sync.dma_start(
                        x0l[:], xtl_d[:, 0:512].rearrange("(es p) j -> p es j", p=128))

                    for dp in range(ET if _PHASES >= 1 else 0):
                        pq = psA.tile([128, NL], F32, tag="acc", name="acc")
                        n_mm = 0
                        for es in range(ET):
                            dcol = slice(dp * 128, (dp + 1) * 128)
                            for lh, rh in ((wqh_sb, xlth_sb), (wqh_sb, xltl_sb),
                                           (wql_sb, xlth_sb)):
                                nc.tensor.matmul(pq[:], lh[:, es, dcol], rh[:, es, :],
                                                 start=(n_mm == 0), stop=(n_mm == 23))
                                n_mm += 1
                        _split(nc, pq[:], qth[dp][:], qtl[dp][:])

                    for ep in range(ET if _PHASES >= 2 else 0):
                        pc = psA.tile([128, NL], F32, tag="acc", name="acc")
                        n_mm = 0
                        for ds in range(ET):
                            ecol = slice(ep * 128, (ep + 1) * 128)
                            for lh, rh in ((wkth_sb, qth), (wkth_sb, qtl),
                                           (wktl_sb, qth)):
                                nc.tensor.matmul(pc[:], lh[:, ds, ecol], rh[ds][:],
                                                 start=(n_mm == 0), stop=(n_mm == 23))
                                n_mm += 1
                        _split(nc, pc[:], cth[ep][:], ctl[ep][:])

            # ---------------- P3: S chunks, chunk-max, fp16 store ----------------
            with tc.tile_pool(name="xhp", bufs=1) as xhp:
                xh_sb = xhp.tile([128, NT // 128, D], F16, tag="xhsb", name="xhsb")

                def p4_stats(m):
                    """global row max of m -> exp-pass bias row."""
                    rowmax = stats.tile([128, 1], F32, tag=f"rmax{m}", name=f"rmax{m}")
                    nc.vector.reduce_max(rowmax[:], pmax[m][:], axis=AX)
                    # bexp = (pmax - rowmax) * SCALE
                    nc.vector.tensor_scalar(bexp[m][:], pmax[m][:], rowmax[:], SCALE,
                                            op0=SUB, op1=MUL)

                def p5_stage(m, jc):
                    """exp -> 4 PE transposes -> one DVE copy; returns P^T tiles."""
                    pch = p5s.tile([128, 512], F16, tag="pch", name="pch")
                    nc.scalar.activation(pch[:], s16[m][:, jc, :], EXP,
                                         bias=bexp[m][:, jc:jc + 1], scale=1.0,
                                         accum_out=esum[m][:, jc:jc + 1])
                    ptt = ptps.tile([128, 4, 128], F16, tag="ptt", name="ptt")
                    for js in range(4):
                        nc.tensor.transpose(ptt[:, js, :],
                                            pch[:, js * 128:(js + 1) * 128], ident16[:])
                    pts = p5s.tile([128, 4, 128], F16, tag="pts", name="pts")
                    nc.vector.tensor_copy(pts[:], ptt[:])
                    return pts

                pts0 = None
                psC = psA
                with (
                    tc.tile_pool(name="stream", bufs=2) as stream,
                ):
                    for jc in range(JC if _PHASES >= 3 else 0):
                        cols = slice(jc * 512, (jc + 1) * 512)
                        if jc == 0:
                            xch, xcl = x0h, x0l
                        else:
                            xch = stream.tile([128, ET, 512], F16, tag="xch", name="xch")
                            xcl = stream.tile([128, ET, 512], F16, tag="xcl", name="xcl")
                            nc.sync.dma_start(
                                xch[:], xth_d[:, cols].rearrange("(es p) j -> p es j", p=128))
                            nc.sync.dma_start(
                                xcl[:], xtl_d[:, cols].rearrange("(es p) j -> p es j", p=128))
                        # V rows for this chunk (used in P5), overlapped here
                        nc.sync.dma_start(
                            xh_sb[:, jc * 4:(jc + 1) * 4, :],
                            xh_d[jc * 512:(jc + 1) * 512, :]
                            .rearrange("(jt p) d -> p jt d", p=128))

                        for m in range(MT):
                            ps = psC.tile([128, 512], F32, tag="acc", name="acc")
                            mcol = slice(m * 128, (m + 1) * 128)
                            n_mm = 0
                            for es in range(ET):
                                for lh, rh in ((cth, xch), (cth, xcl), (ctl, xch)):
                                    nc.tensor.matmul(ps[:], lh[es][:, mcol], rh[:, es, :],
                                                     start=(n_mm == 0), stop=(n_mm == 23))
                                    n_mm += 1
                            nc.vector.reduce_max(pmax[m][:, jc:jc + 1], ps[:], axis=AX)
                            nc.vector.tensor_scalar_mul(
                                bsub[m][:, jc:jc + 1], pmax[m][:, jc:jc + 1], -SCALE)
                            nc.scalar.activation(s16[m][:, jc, :], ps[:], IDENT,
                                                 bias=bsub[m][:, jc:jc + 1], scale=SCALE)
                            if jc == JC - 1 and _PHASES >= 4:
                                # m's chunk stats are complete: fold the P4
                                # bias prep (and the first P5 stage) into the
                                # tail of P3 so P5 starts without a stall
                                p4_stats(m)
                                if m == 0 and _PHASES >= 5:
                                    pts0 = p5_stage(0, 0)

                # ---------------- P5: exp, P^T, P@V, scale ----------------
                with tc.tile_pool(name="pvps", bufs=1,
                                  space=bass.MemorySpace.PSUM) as pvps:
                    # one global software pipeline over (m, jc): stage k+1 is
                    # emitted ahead of chunk k's PV matmuls so the PE never
                    # waits on the DVE PSUM->SBUF copy of its lhsT tiles,
                    # including across m boundaries
                    NS = (MT if _PHASES >= 5 else 0) * JC
                    pv = None
                    pts_cur = pts0
                    for k in range(NS):
                        m, jc = divmod(k, JC)
                        if jc == 0:
                            pv = pvps.tile([128, 2, 512], F32, tag="pv", name="pv")
                        if k + 1 < NS:
                            m1, jc1 = divmod(k + 1, JC)
                            pts_nxt = p5_stage(m1, jc1)
                        else:
                            pts_nxt = None
                        for js in range(4):
                            jt = jc * 4 + js
                            for n in range(2):
                                nc.tensor.matmul(
                                    pv[:, n, :], pts_cur[:, js, :],
                                    xh_sb[:, jt, n * 512:(n + 1) * 512],
                                    start=(jt == 0), stop=(jt == NT // 128 - 1))
                        pts_cur = pts_nxt
                        if jc == JC - 1:
                            rs = stats.tile([128, 1], F32, tag=f"rs{m}", name=f"rs{m}")
                            nc.vector.reduce_sum(rs[:], esum[m][:], axis=AX)
                            nc.vector.reciprocal(recip[:, m:m + 1], rs[:])
                            for n in range(2):
                                osb = p5s.tile([128, 512], F32, tag="osb", name="osb")
                                nc.vector.tensor_scalar_mul(osb[:], pv[:, n, :],
                                                            recip[:, m:m + 1])
                                nc.sync.dma_start(
                                    out_d[m * 128:(m + 1) * 128,
                                          n * 512:(n + 1) * 512], osb[:])

    nc.compile()
    return nc


_NC_CACHE = None


def _split16(a):
    h = a.astype(np.float16)
    l = (a - h.astype(np.float32)).astype(np.float16)
    return h, l


def kernel(inputs, rotation_params, entangle_params):
    global _NC_CACHE
    if _NC_CACHE is None:
        _NC_CACHE = build_nc()
    nc = _NC_CACHE

    x = np.ascontiguousarray(np.asarray(inputs, np.float32))
    wq = np.ascontiguousarray(np.asarray(rotation_params, np.float32))
    wkt = np.ascontiguousarray(np.asarray(entangle_params, np.float32).T)
    xt = np.ascontiguousarray(x.T)

    xh, _ = _split16(x)
    xth, xtl = _split16(xt)
    wqh, wql = _split16(wq)
    wkth, wktl = _split16(wkt)

    in_maps = []
    for c in range(NC):
        cols = slice(c * NL, (c + 1) * NL)
        in_maps.append({
            "xh": xh, "xth": xth, "xtl": xtl,
            "xlth": np.ascontiguousarray(xth[:, cols]),
            "xltl": np.ascontiguousarray(xtl[:, cols]),
            "wqh": wqh, "wql": wql, "wkth": wkth, "wktl": wktl,
        })
    r = run_bass_kernel_spmd(nc, in_maps, list(range(NC)))
    return np.concatenate([r.results[c]["out_local"] for c in range(NC)], axis=0)


# revision 24
# speedup vs baseline: 1.0813x; 1.0053x over previous
"""ClassicalSelfAttention TRN2 kernel — 8-core SPMD, sequence-parallel.

out = softmax((X Wq)(X Wk)^T / sqrt(d)) @ X,  X:[4096,1024] f32, W:[1024,1024].

Per core (rows sharded 8x512), using S_l = ((Xl Wq) Wk^T) X^T so no
replicated projection work:
  Q^T = Wq^T Xl^T        fp16x2 (hh+hl+lh), lhsT = Wq natural layout
  C^T = Wk   Q^T         fp16x2, lhsT = Wk^T (host-transposed)
  S   = C    X^T         fp16x2, rhs = X^T (host-transposed + host-split)
  P   = softmax(S/32)    2-pass; S stored fp16 as (S - chunkmax)*scale
  out = (P @ X) * 1/rowsum   single-pass fp16 (P^T via PE transpose)

Host prep inside kernel(): transpose + fp16 hi/lo split of X / X^T / Wq /
Wk^T (layout-only work; all FLOPs of the computation run on device).
Logits need ~fp32 precision (top-2 gaps down to 3e-3 post-scale; bf16 or
f32r single-pass flips argmax rows), hence fp16x2 for the whole S chain.
P/V tolerate fp16 single-pass. P^T is consumed tile-by-tile straight out
of PSUM copies, so nothing spills to DRAM.

Schedule notes (from TimelineSim traces):
- w1 and w2 share a scope so the Wk^T loads don't WAR-wait on P1's reads
  of the aliased w1 bytes; first S chunk is prefetched the same way.
- P5 runs one global software pipeline over (m, jc): stage k+1 (exp +
  PE transposes + DVE copy) is emitted ahead of chunk k's PV matmuls.
"""
import os
import numpy as np
import concourse.bass as bass
import concourse.bacc as bacc
import concourse.mybir as mybir
import concourse.tile as tile
from concourse import masks
from concourse.bass_utils import run_bass_kernel_spmd

F32 = mybir.dt.float32
F16 = mybir.dt.float16

D = 1024          # embed dim
NT = 4096         # tokens
NC = 8            # cores
NL = NT // NC     # 512 local rows
ET = D // 128     # 8 embed tiles
JC = NT // 512    # 8 j-chunks
MT = NL // 128    # 4 m-tiles
SCALE = float(1.0 / np.sqrt(np.float32(D)))

EXP = mybir.ActivationFunctionType.Exp
IDENT = mybir.ActivationFunctionType.Identity
AX = mybir.AxisListType.X
SUB = mybir.AluOpType.subtract
MUL = mybir.AluOpType.mult

_PHASES = int(os.environ.get("KPHASES", "99"))


def _split(nc, psrc, hdst, ldst):
    """psum f32 -> hdst f16 (round) and ldst f16 (residual), both on DVE."""
    nc.vector.tensor_copy(hdst, psrc)
    nc.vector.tensor_sub(ldst, psrc, hdst)


def build_nc():
    nc = bacc.Bacc("TRN2", target_bir_lowering=False, debug=False)

    xh_d = nc.declare_dram_parameter("xh", [NT, D], F16, isOutput=False)
    xth_d = nc.declare_dram_parameter("xth", [D, NT], F16, isOutput=False)
    xtl_d = nc.declare_dram_parameter("xtl", [D, NT], F16, isOutput=False)
    xlth_d = nc.declare_dram_parameter("xlth", [D, NL], F16, isOutput=False)
    xltl_d = nc.declare_dram_parameter("xltl", [D, NL], F16, isOutput=False)
    wqh_d = nc.declare_dram_parameter("wqh", [D, D], F16, isOutput=False)
    wql_d = nc.declare_dram_parameter("wql", [D, D], F16, isOutput=False)
    wkth_d = nc.declare_dram_parameter("wkth", [D, D], F16, isOutput=False)
    wktl_d = nc.declare_dram_parameter("wktl", [D, D], F16, isOutput=False)
    out_d = nc.declare_dram_parameter("out_local", [NL, D], F32, isOutput=True)

    with tile.TileContext(nc) as tc:
        with (
            tc.tile_pool(name="persist", bufs=1) as persist,
            tc.tile_pool(name="stats", bufs=1) as stats,
            tc.tile_pool(name="ps1", bufs=1, space=bass.MemorySpace.PSUM) as ps1,
        ):
            ident16 = persist.tile([128, 128], F16, tag="id16", name="id16")
            masks.make_identity(nc, ident16[:])

            # chunk max / exp-pass bias / exp sums, one [128, JC] per m-tile
            pmax = [stats.tile([128, JC], F32, tag=f"pmax{m}", name=f"pmax{m}") for m in range(MT)]
            bsub = [stats.tile([128, JC], F32, tag=f"bsub{m}", name=f"bsub{m}") for m in range(MT)]
            bexp = [stats.tile([128, JC], F32, tag=f"bexp{m}", name=f"bexp{m}") for m in range(MT)]
            esum = [stats.tile([128, JC], F32, tag=f"esum{m}", name=f"esum{m}") for m in range(MT)]
            recip = stats.tile([128, MT], F32, tag="recip", name="recip")

            cth = [persist.tile([128, NL], F16, tag=f"cth{e}", name=f"cth{e}")
                   for e in range(ET)]
            ctl = [persist.tile([128, NL], F16, tag=f"ctl{e}", name=f"ctl{e}")
                   for e in range(ET)]
            s16 = [persist.tile([128, JC, 512], F16, tag=f"s16_{m}", name=f"s16_{m}")
                   for m in range(MT)]
            x0h = persist.tile([128, ET, 512], F16, tag="x0h", name="x0h")
            x0l = persist.tile([128, ET, 512], F16, tag="x0l", name="x0l")

            # ---------------- P1 + P2: Q^T then C^T ----------------
            with tc.tile_pool(name="qt", bufs=1) as qtp:
                qth = [qtp.tile([128, NL], F16, tag=f"qth{d}", name=f"qth{d}")
                       for d in range(ET)]
                qtl = [qtp.tile([128, NL], F16, tag=f"qtl{d}", name=f"qtl{d}")
                       for d in range(ET)]

                with (
                    tc.tile_pool(name="psA", bufs=4, space=bass.MemorySpace.PSUM) as psA,
                    tc.tile_pool(name="w1", bufs=1) as w1,
                    tc.tile_pool(name="w2", bufs=1) as w2,
                ):
                    wqh_sb = w1.tile([128, ET, D], F16, tag="wqh", name="wqh")
                    wql_sb = w1.tile([128, ET, D], F16, tag="wql", name="wql")
                    xlth_sb = w1.tile([128, ET, NL], F16, tag="xlth", name="xlth")
                    xltl_sb = w1.tile([128, ET, NL], F16, tag="xltl", name="xltl")
                    wkth_sb = w2.tile([128, ET, D], F16, tag="wkth", name="wkth")
                    wktl_sb = w2.tile([128, ET, D], F16, tag="wktl", name="wktl")

                    # per-es pieces so the first matmuls can start early;
                    # wqh first: the first Ldweights needs it
                    nc.sync.dma_start(wqh_sb[:, 0, 0:512], wqh_d[0:128, 0:512])
                    nc.sync.dma_start(xlth_sb[:, 0, :], xlth_d[0:128, :])
                    nc.sync.dma_start(wqh_sb[:, 0, 512:D], wqh_d[0:128, 512:D])
                    nc.sync.dma_start(wql_sb[:, 0, :], wql_d[0:128, :])
                    nc.sync.dma_start(xltl_sb[:, 0, :], xltl_d[0:128, :])
                    for es in range(1, ET):
                        r = slice(es * 128, (es + 1) * 128)
                        nc.sync.dma_start(wqh_sb[:, es, :], wqh_d[r, :])
                        nc.sync.dma_start(xlth_sb[:, es, :], xlth_d[r, :])
                        nc.sync.dma_start(wql_sb[:, es, :], wql_d[r, :])
                        nc.sync.dma_start(xltl_sb[:, es, :], xltl_d[r, :])
                    # Wk^T and the first S chunk stream in during P1 compute
                    for ds in range(ET):
                        r = slice(ds * 128, (ds + 1) * 128)
                        nc.sync.dma_start(wkth_sb[:, ds, :], wkth_d[r, :])
                        nc.sync.dma_start(wktl_sb[:, ds, :], wktl_d[r, :])
                    nc.sync.dma_start(
                        x0h[:], xth_d[:, 0:512].rearrange("(es p) j -> p es j", p=128))
                    nc.sync.dma_start(
                        x0l[:], xtl_d[:, 0:512].rearrange("(es p) j -> p es j", p=128))

                    for dp in range(ET if _PHASES >= 1 else 0):
                        pq = psA.tile([128, NL], F32, tag="acc", name="acc")
                        n_mm = 0
                        for es in range(ET):
                            dcol = slice(dp * 128, (dp + 1) * 128)
                            for lh, rh in ((wqh_sb, xlth_sb), (wqh_sb, xltl_sb),
                                           (wql_sb, xlth_sb)):
                                nc.tensor.matmul(pq[:], lh[:, es, dcol], rh[:, es, :],
                                                 start=(n_mm == 0), stop=(n_mm == 23))
                                n_mm += 1
                        _split(nc, pq[:], qth[dp][:], qtl[dp][:])

                    for ep in range(ET if _PHASES >= 2 else 0):
                        pc = psA.tile([128, NL], F32, tag="acc", name="acc")
                        n_mm = 0
                        for ds in range(ET):
                            ecol = slice(ep * 128, (ep + 1) * 128)
                            for lh, rh in ((wkth_sb, qth), (wkth_sb, qtl),
                                           (wktl_sb, qth)):
                                nc.tensor.matmul(pc[:], lh[:, ds, ecol], rh[ds][:],
                                                 start=(n_mm == 0), stop=(n_mm == 23))
                                n_mm += 1
                        _split(nc, pc[:], cth[ep][:], ctl[ep][:])

            # ---------------- P3: S chunks, chunk-max, fp16 store ----------------
            with (
                tc.tile_pool(name="xhp", bufs=1) as xhp,
                tc.tile_pool(name="ptps", bufs=2, space=bass.MemorySpace.PSUM) as ptps,
                tc.tile_pool(name="p5s", bufs=3) as p5s,
            ):
                xh_sb = xhp.tile([128, NT // 128, D], F16, tag="xhsb", name="xhsb")

                def p4_stats(m):
                    """global row max of m -> exp-pass bias row."""
                    rowmax = stats.tile([128, 1], F32, tag=f"rmax{m}", name=f"rmax{m}")
                    nc.vector.reduce_max(rowmax[:], pmax[m][:], axis=AX)
                    # bexp = (pmax - rowmax) * SCALE
                    nc.vector.tensor_scalar(bexp[m][:], pmax[m][:], rowmax[:], SCALE,
                                            op0=SUB, op1=MUL)

                def p5_stage(m, jc):
                    """exp -> 4 PE transposes -> one DVE copy; returns P^T tiles."""
                    pch = p5s.tile([128, 512], F16, tag="pch", name="pch")
                    nc.scalar.activation(pch[:], s16[m][:, jc, :], EXP,
                                         bias=bexp[m][:, jc:jc + 1], scale=1.0,
                                         accum_out=esum[m][:, jc:jc + 1])
                    ptt = ptps.tile([128, 4, 128], F16, tag="ptt", name="ptt")
                    for js in range(4):
                        nc.tensor.transpose(ptt[:, js, :],
                                            pch[:, js * 128:(js + 1) * 128], ident16[:])
                    pts = p5s.tile([128, 4, 128], F16, tag="pts", name="pts")
                    nc.vector.tensor_copy(pts[:], ptt[:])
                    return pts

                pts0 = None
                with (
                    tc.tile_pool(name="psC", bufs=3, space=bass.MemorySpace.PSUM) as psC,
                    tc.tile_pool(name="stream", bufs=2) as stream,
                ):
                    for jc in range(JC if _PHASES >= 3 else 0):
                        cols = slice(jc * 512, (jc + 1) * 512)
                        if jc == 0:
                            xch, xcl = x0h, x0l
                        else:
                            xch = stream.tile([128, ET, 512], F16, tag="xch", name="xch")
                            xcl = stream.tile([128, ET, 512], F16, tag="xcl", name="xcl")
                            nc.sync.dma_start(
                                xch[:], xth_d[:, cols].rearrange("(es p) j -> p es j", p=128))
                            nc.sync.dma_start(
                                xcl[:], xtl_d[:, cols].rearrange("(es p) j -> p es j", p=128))
                        # V rows for this chunk (used in P5), overlapped here
                        nc.sync.dma_start(
                            xh_sb[:, jc * 4:(jc + 1) * 4, :],
                            xh_d[jc * 512:(jc + 1) * 512, :]
                            .rearrange("(jt p) d -> p jt d", p=128))

                        for m in range(MT):
                            if jc == 0 and m == 0:
                                ps = ps1.tile([128, 512], F32, tag="acc1", name="acc1")
                            else:
                                ps = psC.tile([128, 512], F32, tag="acc", name="acc")
                            mcol = slice(m * 128, (m + 1) * 128)
                            n_mm = 0
                            for es in range(ET):
                                for lh, rh in ((cth, xch), (cth, xcl), (ctl, xch)):
                                    nc.tensor.matmul(ps[:], lh[es][:, mcol], rh[:, es, :],
                                                     start=(n_mm == 0), stop=(n_mm == 23))
                                    n_mm += 1
                            nc.vector.reduce_max(pmax[m][:, jc:jc + 1], ps[:], axis=AX)
                            nc.vector.tensor_scalar_mul(
                                bsub[m][:, jc:jc + 1], pmax[m][:, jc:jc + 1], -SCALE)
                            nc.scalar.activation(s16[m][:, jc, :], ps[:], IDENT,
                                                 bias=bsub[m][:, jc:jc + 1], scale=SCALE)
                            if jc == JC - 1 and _PHASES >= 4:
                                # m's chunk stats are complete: fold the P4
                                # bias prep (and the first P5 stage) into the
                                # tail of P3 so P5 starts without a stall
                                p4_stats(m)
                                if m == 0 and _PHASES >= 5:
                                    pts0 = p5_stage(0, 0)

                # ---------------- P5: exp, P^T, P@V, scale ----------------
                with tc.tile_pool(name="pvps", bufs=2,
                                  space=bass.MemorySpace.PSUM) as pvps:
                    # one global software pipeline over (m, jc): stage k+1 is
                    # emitted ahead of chunk k's PV matmuls so the PE never
                    # waits on the DVE PSUM->SBUF copy of its lhsT tiles,
                    # including across m boundaries
                    NS = (MT if _PHASES >= 5 else 0) * JC
                    pv = None
                    pts_cur = pts0
                    for k in range(NS):
                        m, jc = divmod(k, JC)
                        if jc == 0:
                            pv = pvps.tile([128, 2, 512], F32, tag="pv", name="pv")
                        if k + 1 < NS:
                            m1, jc1 = divmod(k + 1, JC)
                            pts_nxt = p5_stage(m1, jc1)
                        else:
                            pts_nxt = None
                        for js in range(4):
                            jt = jc * 4 + js
                            for n in range(2):
                                nc.tensor.matmul(
                                    pv[:, n, :], pts_cur[:, js, :],
                                    xh_sb[:, jt, n * 512:(n + 1) * 512],
                                    start=(jt == 0), stop=(jt == NT // 128 - 1))
                        pts_cur = pts_nxt
                        if jc == JC - 1:
                            rs = stats.tile([128, 1], F32, tag=f"rs{m}", name=f"rs{m}")
                            nc.vector.reduce_sum(rs[:], esum[m][:], axis=AX)
                            nc.vector.reciprocal(recip[:, m:m + 1], rs[:])
                            for n in range(2):
                                osb = p5s.tile([128, 512], F32, tag="osb", name="osb")
                                if n == 0:
                                    # ACT and DVE scale the two halves in parallel
                                    nc.scalar.activation(osb[:], pv[:, n, :],
                                                         mybir.ActivationFunctionType.Copy,
                                                         scale=recip[:, m:m + 1])
                                else:
                                    nc.vector.tensor_scalar_mul(osb[:], pv[:, n, :],
                                                                recip[:, m:m + 1])
                                nc.sync.dma_start(
                                    out_d[m * 128:(m + 1) * 128,
                                          n * 512:(n + 1) * 512], osb[:])

    nc.compile()
    return nc


_NC_CACHE = None


def _split16(a):
    h = a.astype(np.float16)
    l = (a - h.astype(np.float32)).astype(np.float16)
    return h, l


def kernel(inputs, rotation_params, entangle_params):
    global _NC_CACHE
    if _NC_CACHE is None:
        _NC_CACHE = build_nc()
    nc = _NC_CACHE

    x = np.ascontiguousarray(np.asarray(inputs, np.float32))
    wq = np.ascontiguousarray(np.asarray(rotation_params, np.float32))
    wkt = np.ascontiguousarray(np.asarray(entangle_params, np.float32).T)
    xt = np.ascontiguousarray(x.T)

    xh, _ = _split16(x)
    xth, xtl = _split16(xt)
    wqh, wql = _split16(wq)
    wkth, wktl = _split16(wkt)

    in_maps = []
    for c in range(NC):
        cols = slice(c * NL, (c + 1) * NL)
        in_maps.append({
            "xh": xh, "xth": xth, "xtl": xtl,
            "xlth": np.ascontiguousarray(xth[:, cols]),
            "xltl": np.ascontiguousarray(xtl[:, cols]),
            "wqh": wqh, "wql": wql, "wkth": wkth, "wktl": wktl,
        })
    r = run_bass_kernel_spmd(nc, in_maps, list(range(NC)))
    return np.concatenate([r.results[c]["out_local"] for c in range(NC)], axis=0)


# revision 28
# speedup vs baseline: 1.0841x; 1.0026x over previous
"""ClassicalSelfAttention TRN2 kernel — 8-core SPMD, sequence-parallel.

out = softmax((X Wq)(X Wk)^T / sqrt(d)) @ X,  X:[4096,1024] f32, W:[1024,1024].

Per core (rows sharded 8x512), using S_l = ((Xl Wq) Wk^T) X^T so no
replicated projection work:
  Q^T = Wq^T Xl^T        fp16x2 (hh+hl+lh), lhsT = Wq natural layout
  C^T = Wk   Q^T         fp16x2, lhsT = Wk^T (host-transposed)
  S   = C    X^T         fp16x2, rhs = X^T (host-transposed + host-split)
  P   = softmax(S/32)    2-pass; S stored fp16 as (S - chunkmax)*scale
  out = (P @ X) * 1/rowsum   single-pass fp16 (P^T via PE transpose)

Host prep inside kernel(): transpose + fp16 hi/lo split of X / X^T / Wq /
Wk^T (layout-only work; all FLOPs of the computation run on device).
Logits need ~fp32 precision (top-2 gaps down to 3e-3 post-scale; bf16 or
f32r single-pass flips argmax rows), hence fp16x2 for the whole S chain.
P/V tolerate fp16 single-pass. P^T is consumed tile-by-tile straight out
of PSUM copies, so nothing spills to DRAM.

Schedule notes (from TimelineSim traces):
- w1 and w2 share a scope so the Wk^T loads don't WAR-wait on P1's reads
  of the aliased w1 bytes; first S chunk is prefetched the same way.
- P5 runs one global software pipeline over (m, jc): stage k+1 (exp +
  PE transposes + DVE copy) is emitted ahead of chunk k's PV matmuls.
"""
import os
import numpy as np
import concourse.bass as bass
import concourse.bacc as bacc
import concourse.mybir as mybir
import concourse.tile as tile
from concourse import masks
from concourse.bass_utils import run_bass_kernel_spmd

F32 = mybir.dt.float32
F16 = mybir.dt.float16

D = 1024          # embed dim
NT = 4096         # tokens
NC = 8            # cores
NL = NT // NC     # 512 local rows
ET = D // 128     # 8 embed tiles
JC = NT // 512    # 8 j-chunks
MT = NL // 128    # 4 m-tiles
SCALE = float(1.0 / np.sqrt(np.float32(D)))

EXP = mybir.ActivationFunctionType.Exp
IDENT = mybir.ActivationFunctionType.Identity
AX = mybir.AxisListType.X
SUB = mybir.AluOpType.subtract
MUL = mybir.AluOpType.mult

_PHASES = int(os.environ.get("KPHASES", "99"))


def _split(nc, psrc, hdst, ldst):
    """psum f32 -> hdst f16 (round) and ldst f16 (residual), both on DVE."""
    nc.vector.tensor_copy(hdst, psrc)
    nc.vector.tensor_sub(ldst, psrc, hdst)


def build_nc():
    nc = bacc.Bacc("TRN2", target_bir_lowering=False, debug=False)

    xh_d = nc.declare_dram_parameter("xh", [NT, D], F16, isOutput=False)
    xth_d = nc.declare_dram_parameter("xth", [D, NT], F16, isOutput=False)
    xtl_d = nc.declare_dram_parameter("xtl", [D, NT], F16, isOutput=False)
    xlth_d = nc.declare_dram_parameter("xlth", [D, NL], F16, isOutput=False)
    xltl_d = nc.declare_dram_parameter("xltl", [D, NL], F16, isOutput=False)
    wqh_d = nc.declare_dram_parameter("wqh", [D, D], F16, isOutput=False)
    wql_d = nc.declare_dram_parameter("wql", [D, D], F16, isOutput=False)
    wkth_d = nc.declare_dram_parameter("wkth", [D, D], F16, isOutput=False)
    wktl_d = nc.declare_dram_parameter("wktl", [D, D], F16, isOutput=False)
    out_d = nc.declare_dram_parameter("out_local", [NL, D], F32, isOutput=True)

    with tile.TileContext(nc) as tc:
        with (
            tc.tile_pool(name="persist", bufs=1) as persist,
            tc.tile_pool(name="stats", bufs=1) as stats,
            tc.tile_pool(name="ps1", bufs=1, space=bass.MemorySpace.PSUM) as ps1,
        ):
            ident16 = persist.tile([128, 128], F16, tag="id16", name="id16")
            masks.make_identity(nc, ident16[:])

            # chunk max / exp-pass bias / exp sums, one [128, JC] per m-tile
            pmax = [stats.tile([128, JC], F32, tag=f"pmax{m}", name=f"pmax{m}") for m in range(MT)]
            bsub = [stats.tile([128, JC], F32, tag=f"bsub{m}", name=f"bsub{m}") for m in range(MT)]
            bexp = [stats.tile([128, JC], F32, tag=f"bexp{m}", name=f"bexp{m}") for m in range(MT)]
            esum = [stats.tile([128, JC], F32, tag=f"esum{m}", name=f"esum{m}") for m in range(MT)]
            recip = stats.tile([128, MT], F32, tag="recip", name="recip")

            cth = [persist.tile([128, NL], F16, tag=f"cth{e}", name=f"cth{e}")
                   for e in range(ET)]
            ctl = [persist.tile([128, NL], F16, tag=f"ctl{e}", name=f"ctl{e}")
                   for e in range(ET)]
            s16 = [persist.tile([128, JC, 512], F16, tag=f"s16_{m}", name=f"s16_{m}")
                   for m in range(MT)]
            x0h = persist.tile([128, ET, 512], F16, tag="x0h", name="x0h")
            x0l = persist.tile([128, ET, 512], F16, tag="x0l", name="x0l")

            # ---------------- P1 + P2: Q^T then C^T ----------------
            with tc.tile_pool(name="qt", bufs=1) as qtp:
                qth = [qtp.tile([128, NL], F16, tag=f"qth{d}", name=f"qth{d}")
                       for d in range(ET)]
                qtl = [qtp.tile([128, NL], F16, tag=f"qtl{d}", name=f"qtl{d}")
                       for d in range(ET)]

                with (
                    tc.tile_pool(name="psA", bufs=4, space=bass.MemorySpace.PSUM) as psA,
                    tc.tile_pool(name="w1", bufs=1) as w1,
                    tc.tile_pool(name="w2", bufs=1) as w2,
                ):
                    wqh_sb = w1.tile([128, ET, D], F16, tag="wqh", name="wqh")
                    wql_sb = w1.tile([128, ET, D], F16, tag="wql", name="wql")
                    xlth_sb = w1.tile([128, ET, NL], F16, tag="xlth", name="xlth")
                    xltl_sb = w1.tile([128, ET, NL], F16, tag="xltl", name="xltl")
                    wkth_sb = w2.tile([128, ET, D], F16, tag="wkth", name="wkth")
                    wktl_sb = w2.tile([128, ET, D], F16, tag="wktl", name="wktl")

                    # per-es pieces so the first matmuls can start early;
                    # wqh first: the first Ldweights needs it
                    nc.sync.dma_start(wqh_sb[:, 0, 0:512], wqh_d[0:128, 0:512])
                    nc.sync.dma_start(xlth_sb[:, 0, :], xlth_d[0:128, :])
                    nc.sync.dma_start(wqh_sb[:, 0, 512:D], wqh_d[0:128, 512:D])
                    nc.sync.dma_start(wql_sb[:, 0, :], wql_d[0:128, :])
                    nc.sync.dma_start(xltl_sb[:, 0, :], xltl_d[0:128, :])
                    for es in range(1, ET):
                        r = slice(es * 128, (es + 1) * 128)
                        nc.sync.dma_start(wqh_sb[:, es, :], wqh_d[r, :])
                        nc.sync.dma_start(xlth_sb[:, es, :], xlth_d[r, :])
                        nc.sync.dma_start(wql_sb[:, es, :], wql_d[r, :])
                        nc.sync.dma_start(xltl_sb[:, es, :], xltl_d[r, :])
                    # Wk^T and the first S chunk stream in during P1 compute
                    for ds in range(ET):
                        r = slice(ds * 128, (ds + 1) * 128)
                        nc.sync.dma_start(wkth_sb[:, ds, :], wkth_d[r, :])
                        nc.sync.dma_start(wktl_sb[:, ds, :], wktl_d[r, :])
                    nc.sync.dma_start(
                        x0h[:], xth_d[:, 0:512].rearrange("(es p) j -> p es j", p=128))
                    nc.sync.dma_start(
                        x0l[:], xtl_d[:, 0:512].rearrange("(es p) j -> p es j", p=128))

                    for dp in range(ET if _PHASES >= 1 else 0):
                        pq = psA.tile([128, NL], F32, tag="acc", name="acc")
                        n_mm = 0
                        for es in range(ET):
                            dcol = slice(dp * 128, (dp + 1) * 128)
                            for lh, rh in ((wqh_sb, xlth_sb), (wqh_sb, xltl_sb),
                                           (wql_sb, xlth_sb)):
                                nc.tensor.matmul(pq[:], lh[:, es, dcol], rh[:, es, :],
                                                 start=(n_mm == 0), stop=(n_mm == 23))
                                n_mm += 1
                        _split(nc, pq[:], qth[dp][:], qtl[dp][:])

                    for ep in range(ET if _PHASES >= 2 else 0):
                        pc = psA.tile([128, NL], F32, tag="acc", name="acc")
                        n_mm = 0
                        for ds in range(ET):
                            ecol = slice(ep * 128, (ep + 1) * 128)
                            for lh, rh in ((wkth_sb, qth), (wkth_sb, qtl),
                                           (wktl_sb, qth)):
                                nc.tensor.matmul(pc[:], lh[:, ds, ecol], rh[ds][:],
                                                 start=(n_mm == 0), stop=(n_mm == 23))
                                n_mm += 1
                        _split(nc, pc[:], cth[ep][:], ctl[ep][:])

            # ---------------- P3: S chunks, chunk-max, fp16 store ----------------
            with (
                tc.tile_pool(name="xhp", bufs=1) as xhp,
                tc.tile_pool(name="ptps", bufs=2, space=bass.MemorySpace.PSUM) as ptps,
                tc.tile_pool(name="p5s", bufs=3) as p5s,
            ):
                xh_sb = xhp.tile([128, NT // 128, D], F16, tag="xhsb", name="xhsb")

                def p4_stats(m):
                    """global row max of m -> exp-pass bias row."""
                    rowmax = stats.tile([128, 1], F32, tag=f"rmax{m}", name=f"rmax{m}")
                    nc.vector.reduce_max(rowmax[:], pmax[m][:], axis=AX)
                    # bexp = (pmax - rowmax) * SCALE
                    nc.vector.tensor_scalar(bexp[m][:], pmax[m][:], rowmax[:], SCALE,
                                            op0=SUB, op1=MUL)

                def p5_stage(m, jc, pool=None, tag="pts"):
                    """exp -> 4 PE transposes -> one DVE copy; returns P^T tiles."""
                    pch = p5s.tile([128, 512], F16, tag="pch", name="pch")
                    nc.scalar.activation(pch[:], s16[m][:, jc, :], EXP,
                                         bias=bexp[m][:, jc:jc + 1], scale=1.0,
                                         accum_out=esum[m][:, jc:jc + 1])
                    ptt = ptps.tile([128, 4, 128], F16, tag="ptt", name="ptt")
                    for js in range(4):
                        nc.tensor.transpose(ptt[:, js, :],
                                            pch[:, js * 128:(js + 1) * 128], ident16[:])
                    pts = (pool or p5s).tile([128, 4, 128], F16, tag=tag, name=tag)
                    nc.vector.tensor_copy(pts[:], ptt[:])
                    return pts

                pts0 = None
                with (
                    tc.tile_pool(name="psC", bufs=3, space=bass.MemorySpace.PSUM) as psC,
                    tc.tile_pool(name="stream", bufs=2) as stream,
                ):
                    for jc in range(JC if _PHASES >= 3 else 0):
                        cols = slice(jc * 512, (jc + 1) * 512)
                        if jc == 0:
                            xch, xcl = x0h, x0l
                        else:
                            xch = stream.tile([128, ET, 512], F16, tag="xch", name="xch")
                            xcl = stream.tile([128, ET, 512], F16, tag="xcl", name="xcl")
                            nc.sync.dma_start(
                                xch[:], xth_d[:, cols].rearrange("(es p) j -> p es j", p=128))
                            nc.sync.dma_start(
                                xcl[:], xtl_d[:, cols].rearrange("(es p) j -> p es j", p=128))
                        # V rows for this chunk (used in P5), overlapped here
                        nc.sync.dma_start(
                            xh_sb[:, jc * 4:(jc + 1) * 4, :],
                            xh_d[jc * 512:(jc + 1) * 512, :]
                            .rearrange("(jt p) d -> p jt d", p=128))

                        for m in range(MT):
                            if jc == 0 and m == 0:
                                ps = ps1.tile([128, 512], F32, tag="acc1", name="acc1")
                            else:
                                ps = psC.tile([128, 512], F32, tag="acc", name="acc")
                            mcol = slice(m * 128, (m + 1) * 128)
                            n_mm = 0
                            for es in range(ET):
                                for lh, rh in ((cth, xch), (cth, xcl), (ctl, xch)):
                                    nc.tensor.matmul(ps[:], lh[es][:, mcol], rh[:, es, :],
                                                     start=(n_mm == 0), stop=(n_mm == 23))
                                    n_mm += 1
                            nc.vector.reduce_max(pmax[m][:, jc:jc + 1], ps[:], axis=AX)
                            nc.vector.tensor_scalar_mul(
                                bsub[m][:, jc:jc + 1], pmax[m][:, jc:jc + 1], -SCALE)
                            nc.scalar.activation(s16[m][:, jc, :], ps[:], IDENT,
                                                 bias=bsub[m][:, jc:jc + 1], scale=SCALE)
                            if jc == JC - 1 and _PHASES >= 4:
                                # m's chunk stats are complete: fold the P4
                                # bias prep (and the first P5 stage) into the
                                # tail of P3 so P5 starts without a stall
                                p4_stats(m)
                                if m == 0 and _PHASES >= 5:
                                    pts0 = p5_stage(0, 0)

                # ---------------- P5: exp, P^T, P@V, scale ----------------
                with (
                    tc.tile_pool(name="pvps", bufs=2,
                                 space=bass.MemorySpace.PSUM) as pvps,
                    tc.tile_pool(name="ptlast", bufs=1) as ptlast,
                ):
                    # one global software pipeline over (m, jc): stage k+1 is
                    # emitted ahead of chunk k's PV matmuls so the PE never
                    # waits on the DVE PSUM->SBUF copy of its lhsT tiles,
                    # including across m boundaries
                    NS = (MT if _PHASES >= 5 else 0) * JC
                    pv = None
                    pts_cur = pts0
                    last_pts = []

                    def out_half(m, pv, n, on_act):
                        osb = p5s.tile([128, 512], F32, tag="osb", name="osb")
                        if on_act:
                            nc.scalar.activation(osb[:], pv[n][:],
                                                 mybir.ActivationFunctionType.Copy,
                                                 scale=recip[:, m:m + 1])
                        else:
                            nc.vector.tensor_scalar_mul(osb[:], pv[n][:],
                                                        recip[:, m:m + 1])
                        nc.sync.dma_start(
                            out_d[m * 128:(m + 1) * 128,
                                  n * 512:(n + 1) * 512], osb[:])

                    for k in range(NS):
                        m, jc = divmod(k, JC)
                        last_m = m == MT - 1
                        if jc == 0:
                            pv = [pvps.tile([128, 512], F32, tag="pv0", name="pv0"),
                                  pvps.tile([128, 512], F32, tag="pv1", name="pv1")]
                        if k + 1 < NS:
                            m1, jc1 = divmod(k + 1, JC)
                            if m1 == MT - 1:
                                pts_nxt = p5_stage(m1, jc1, pool=ptlast,
                                                   tag=f"ptsL{jc1}")
                            else:
                                pts_nxt = p5_stage(m1, jc1)
                        else:
                            pts_nxt = None
                        if last_m:
                            last_pts.append(pts_cur)
                        # the last m runs n=0 only here; its n=1 pass follows
                        # after, so the n=0 output store overlaps n=1 matmuls
                        for js in range(4):
                            jt = jc * 4 + js
                            for n in ((0,) if last_m else (0, 1)):
                                nc.tensor.matmul(
                                    pv[n][:], pts_cur[:, js, :],
                                    xh_sb[:, jt, n * 512:(n + 1) * 512],
                                    start=(jt == 0), stop=(jt == NT // 128 - 1))
                        pts_cur = pts_nxt
                        if jc == JC - 1:
                            rs = stats.tile([128, 1], F32, tag=f"rs{m}", name=f"rs{m}")
                            nc.vector.reduce_sum(rs[:], esum[m][:], axis=AX)
                            nc.vector.reciprocal(recip[:, m:m + 1], rs[:])
                            if not last_m:
                                out_half(m, pv, 0, on_act=True)
                                out_half(m, pv, 1, on_act=False)
                    if NS:
                        m = MT - 1
                        out_half(m, pv, 0, on_act=True)
                        for jc in range(JC):
                            for js in range(4):
                                jt = jc * 4 + js
                                nc.tensor.matmul(
                                    pv[1][:], last_pts[jc][:, js, :],
                                    xh_sb[:, jt, 512:1024],
                                    start=(jt == 0), stop=(jt == NT // 128 - 1))
                        out_half(m, pv, 1, on_act=False)

    nc.compile()
    return nc


_NC_CACHE = None


def _split16(a):
    h = a.astype(np.float16)
    l = (a - h.astype(np.float32)).astype(np.float16)
    return h, l


def kernel(inputs, rotation_params, entangle_params):
    global _NC_CACHE
    if _NC_CACHE is None:
        _NC_CACHE = build_nc()
    nc = _NC_CACHE

    x = np.ascontiguousarray(np.asarray(inputs, np.float32))
    wq = np.ascontiguousarray(np.asarray(rotation_params, np.float32))
    wkt = np.ascontiguousarray(np.asarray(entangle_params, np.float32).T)
    xt = np.ascontiguousarray(x.T)

    xh, _ = _split16(x)
    xth, xtl = _split16(xt)
    wqh, wql = _split16(wq)
    wkth, wktl = _split16(wkt)

    in_maps = []
    for c in range(NC):
        cols = slice(c * NL, (c + 1) * NL)
        in_maps.append({
            "xh": xh, "xth": xth, "xtl": xtl,
            "xlth": np.ascontiguousarray(xth[:, cols]),
            "xltl": np.ascontiguousarray(xtl[:, cols]),
            "wqh": wqh, "wql": wql, "wkth": wkth, "wktl": wktl,
        })
    r = run_bass_kernel_spmd(nc, in_maps, list(range(NC)))
    return np.concatenate([r.results[c]["out_local"] for c in range(NC)], axis=0)
